# revision 1
# baseline (speedup 1.0000x reference)
"""Trainium2 Bass kernel: transformer encoder layer (S=4096,B=2,D=512,H=8,F=2048),
causal attention + RoPE, distributed over 8 NeuronCores.

Sharding (SPMD: one program, per-core data):
  - LN1+RoPE: sequence-parallel (core c owns s in [512c, 512(c+1)), both batches)
  - QKV projections: token-parallel (each core projects q,k,v of ALL heads for
    its own tokens, emitting q/k head-major and v token-major directly)
  - AllToAll(qkv)             [384KB/rank] -> core c gets head c for all tokens
  - causal attention: head-parallel (core c owns head c, full S, both b)
  - AllToAll(attn_head^T)     [512KB/rank, per batch] -> all heads, own tokens
  - out_proj + residual + LN2 + FFN: token-parallel (core c owns its s-slice)
LayerNorm affine params are folded into downstream weights host-side.
Softmax denominators come free from a ones-column appended to V.
All weights preloaded to SBUF up front so their DMA overlaps early compute.
"""
import numpy as np
import ml_dtypes
from contextlib import ExitStack

import concourse.bass as bass
import concourse.tile as tile
from concourse import bacc, mybir
from concourse.bass_utils import run_bass_kernel_spmd
from concourse.masks import make_identity

F32 = mybir.dt.float32
F32R = mybir.dt.float32r
BF16 = mybir.dt.bfloat16
AF = mybir.ActivationFunctionType
ALU = mybir.AluOpType

S, B, D, H, Dh, F = 4096, 2, 512, 8, 64, 2048
W = 8                    # cores
SL = S // W              # 512 s-positions per core
TL = SL * B              # 1024 local tokens
EPS = 1e-5
SCALE = 1.0 / float(np.sqrt(Dh))  # 0.125

NT = TL // 128           # 8 local token tiles
NK = D // 128            # 4 contraction chunks over D
NF = F // 128            # 16 chunks over F
NS = S // 128            # 32 key tiles per batch
QKC = 2 * Dh * H         # 1024 packed q,k output columns (h-major)
CCW = QKC + NT * Dh      # 1536 columns per A2A block

_NC_CACHE = {}
_GELU_OVERRIDE = None  # set to AF.Identity in sim tests (CoreSim lacks Gelu)


def _layer_norm_stats(nc, pool, x_t, eps_sb):
    """Returns (rstd [128,1], negmean_rstd [128,1]) for rows of x_t."""
    stats = pool.tile([128, 6], F32, tag="st")
    nc.vector.bn_stats(out=stats, in_=x_t)
    mv = pool.tile([128, 2], F32, tag="mv")
    nc.vector.bn_aggr(out=mv, in_=stats)
    sd = pool.tile([128, 1], F32, tag="sd")
    nc.scalar.activation(out=sd, in_=mv[:, 1:2], func=AF.Sqrt, bias=eps_sb)
    rstd = pool.tile([128, 1], F32, tag="rs")
    nc.vector.reciprocal(out=rstd, in_=sd)
    nm = pool.tile([128, 1], F32, tag="nm")
    nc.vector.tensor_mul(nm, mv[:, 0:1], rstd)
    nc.vector.tensor_scalar_mul(nm, nm, -1.0)
    return rstd, nm


def _build_nc(flags, n_reps=1):
    """flags = (has_ropeb, has_bqk, has_bv, has_bo, has_b2)

    n_reps > 1 builds a timing variant with the body unrolled n_reps times
    (same I/O, idempotent) so device time can be read off the slope.
    """
    import os as _os
    has_ropeb, has_bqk, has_bv, has_bo, has_b2 = flags
    skip_cc = bool(int(_os.environ.get("K_SKIP_CC", "0")))
    max_phase = int(_os.environ.get("K_MAX_PHASE", "7"))
    nc = bacc.Bacc("TRN2", target_bir_lowering=False, debug=False, num_devices=W)

    # ---- I/O ----
    src_loc = nc.dram_tensor("src_loc", [TL, D], F32, kind="ExternalInput")
    cosw = nc.dram_tensor("cosw", [SL, D], F32, kind="ExternalInput")
    rotw = nc.dram_tensor("rotw", [SL, D], F32, kind="ExternalInput")
    ropeb = nc.dram_tensor("ropeb", [SL, D], F32, kind="ExternalInput") if has_ropeb else None
    wqk_t = nc.dram_tensor("wqk_t", [D, QKC], BF16, kind="ExternalInput")
    wv_t = nc.dram_tensor("wv_t", [D, D], BF16, kind="ExternalInput")
    bqk = nc.dram_tensor("bqk", [128, H], F32, kind="ExternalInput")
    bv = nc.dram_tensor("bv", [D], F32, kind="ExternalInput")
    wo_t = nc.dram_tensor("wo_t", [D, D], BF16, kind="ExternalInput")
    bo = nc.dram_tensor("bo", [D], F32, kind="ExternalInput")
    w1_t = nc.dram_tensor("w1_t", [D, F], F32R, kind="ExternalInput")
    b1p = nc.dram_tensor("b1p", [F], F32, kind="ExternalInput")
    w2_t = nc.dram_tensor("w2_t", [F, D], F32R, kind="ExternalInput")
    b2 = nc.dram_tensor("b2", [D], F32, kind="ExternalInput")
    out_loc = nc.dram_tensor("out_loc", [TL, D], F32, kind="ExternalOutput")

    with tile.TileContext(nc) as tc:
      for _rep in range(n_reps):
       with ExitStack() as top:
        dram = top.enter_context(tc.tile_pool(name="dram", bufs=1, space="DRAM"))
        consts = top.enter_context(tc.tile_pool(name="consts", bufs=1))

        # ---------- constants + all weights up front ----------
        ident = consts.tile([128, 128], F32)
        make_identity(nc, ident)
        # causal diag masks: masks[:, j, q] = 1.0 if q >= k + j*128 else 0.0
        masks = consts.tile([128, 4, 512], BF16)
        for j in range(4):
            nc.gpsimd.memset(masks[:, j, :], 1.0)
            nc.gpsimd.affine_select(
                out=masks[:, j, :], in_=masks[:, j, :],
                compare_op=ALU.is_ge, fill=0.0,
                base=-j * 128, channel_multiplier=-1, pattern=[[1, 512]],
            )
        eps_sb = consts.tile([128, 1], F32)
        nc.vector.memset(eps_sb, EPS)
        ones_col = consts.tile([65, Dh], F32)
        nc.vector.memset(ones_col, 1.0)
        ones_row = ones_col[64:65, :]   # base partition 64, matches pa_sb row
        # weight SBUF tiles declared here, DMAs emitted later (so the P1/P2'
        # token loads aren't queued behind 9MB of weight traffic)
        wqk_sb = consts.tile([128, NK, QKC], BF16)
        wv_sb = consts.tile([128, NK, D], BF16)
        wo_sb = consts.tile([128, NK, D], BF16)
        w1_sb = consts.tile([128, NK, F], F32R)
        w2_sb = consts.tile([128, NF, D], F32R)
        b1_sb = consts.tile([128, NF], F32)
        bqk_sb = consts.tile([128, H], F32)
        if has_bqk:
            nc.sync.dma_start(out=bqk_sb, in_=bqk)
        bv_bc = consts.tile([128, D], F32)
        if has_bv:
            bv_row = consts.tile([1, D], F32)
            nc.sync.dma_start(out=bv_row, in_=bv[None, :])
            nc.gpsimd.partition_broadcast(bv_bc, bv_row)
        bo_bc = consts.tile([128, D], F32)
        if has_bo:
            bo_row = consts.tile([1, D], F32)
            nc.sync.dma_start(out=bo_row, in_=bo[None, :])
            nc.gpsimd.partition_broadcast(bo_bc, bo_row)
        b2_bc = consts.tile([128, D], F32)
        if has_b2:
            b2_row = consts.tile([1, D], F32)
            nc.sync.dma_start(out=b2_row, in_=b2[None, :])
            nc.gpsimd.partition_broadcast(b2_bc, b2_row)

        # collective buffers (qkv exchange split per batch so b0's AllToAll
        # overlaps b1's LN/RoPE/projection compute)
        CCWB = SL + 4 * Dh   # 768 columns per block: qk | v
        ccq_in = [dram.tile([W, 128, CCWB], BF16, name=f"ccq_in_{b}")
                  for b in range(B)]
        ccq_out = [dram.tile([W, 128, CCWB], BF16, name=f"ccq_out_{b}")
                   for b in range(B)]
        cc2_in = [dram.tile([W, Dh, SL], BF16, name=f"cc2_in_{b}") for b in range(B)]
        cc2_out = [dram.tile([W, Dh, SL], BF16, name=f"cc2_out_{b}") for b in range(B)]

        # ============ P1: LN1 + RoPE + transpose (seq-parallel) ============
        # ============ P2': QKV for own tokens, all heads ============
        with ExitStack() as ctx:
            xt_pool = ctx.enter_context(tc.tile_pool(name="xT", bufs=1))
            # D-major LN1/RoPE outputs (live through P2' only)
            xrT = xt_pool.tile([128, NK, TL], BF16)
            xnT = xt_pool.tile([128, NK, TL], BF16)
            sb = ctx.enter_context(tc.tile_pool(name="p1", bufs=3))
            small = ctx.enter_context(tc.tile_pool(name="p1s", bufs=4))
            ropet = ctx.enter_context(tc.tile_pool(name="p1r", bufs=1))
            cosw_all = ropet.tile([128, 4, D], F32)
            nc.sync.dma_start(out=cosw_all,
                              in_=cosw.rearrange("(s p) d -> p s d", p=128))
            rotw_all = ropet.tile([128, 4, D], F32)
            nc.sync.dma_start(out=rotw_all,
                              in_=rotw.rearrange("(s p) d -> p s d", p=128))
            trps = ctx.enter_context(tc.tile_pool(name="p1ps", bufs=2, space="PSUM"))
            qkps = ctx.enter_context(tc.tile_pool(name="p2qk", bufs=2, space="PSUM"))
            vps = ctx.enter_context(tc.tile_pool(name="p2v", bufs=2, space="PSUM"))
            stg = ctx.enter_context(tc.tile_pool(name="p2stg", bufs=3))

            def p1_tile(t):
                s_t = sb.tile([128, D], F32, tag="s")
                nc.sync.dma_start(out=s_t, in_=src_loc[t * 128:(t + 1) * 128, :])
                rstd, nm = _layer_norm_stats(nc, small, s_t, eps_sb)
                xn_t = sb.tile([128, D], F32, tag="xn")
                nc.vector.tensor_scalar(
                    out=xn_t, in0=s_t, scalar1=rstd, scalar2=nm,
                    op0=ALU.mult, op1=ALU.add,
                )
                # RoPE (ln1 affine folded into cosw/rotw/ropeb host-side)
                sc = t % (SL // 128)
                cosw_t = cosw_all[:, sc, :]
                rotw_t = rotw_all[:, sc, :]
                xr_t = sb.tile([128, D], F32, tag="xr")
                rt = sb.tile([128, D], F32, tag="rt")
                xnv = xn_t.rearrange("p (h i two) -> p h i two", h=H, two=2)
                rtv = rt.rearrange("p (h d) -> p h d", h=H)
                rwv = rotw_t.rearrange("p (h d) -> p h d", h=H)
                # rt[:, :, :32] = xn[:, :, 1::2] * rotw[:, :, :32]
                nc.vector.tensor_mul(rtv[:, :, 0:32], xnv[:, :, :, 1], rwv[:, :, 0:32])
                # rt[:, :, 32:] = xn[:, :, 0::2] * rotw[:, :, 32:]
                nc.vector.tensor_mul(rtv[:, :, 32:64], xnv[:, :, :, 0], rwv[:, :, 32:64])
                nc.vector.tensor_mul(xr_t, xn_t, cosw_t)
                nc.vector.tensor_add(xr_t, xr_t, rt)
                if has_ropeb:
                    rb_t = sb.tile([128, D], F32, tag="rb")
                    nc.sync.dma_start(out=rb_t, in_=ropeb[sc * 128:(sc + 1) * 128, :])
                    nc.vector.tensor_add(xr_t, xr_t, rb_t)
                # transpose both to D-major; xr -> DVE copy, xn -> ScalarE copy
                ps_r = trps.tile([128, 512], F32, tag="trr")
                ps_n = trps.tile([128, 512], F32, tag="trn")
                for k in range(NK):
                    nc.tensor.transpose(ps_r[:, k * 128:(k + 1) * 128],
                                        xr_t[:, k * 128:(k + 1) * 128], ident)
                    nc.tensor.transpose(ps_n[:, k * 128:(k + 1) * 128],
                                        xn_t[:, k * 128:(k + 1) * 128], ident)
                nc.vector.tensor_copy(
                    xrT[:, :, t * 128:(t + 1) * 128],
                    ps_r.rearrange("p (k i) -> p k i", k=NK))
                nc.scalar.copy(
                    out=xnT[:, :, t * 128:(t + 1) * 128],
                    in_=ps_n.rearrange("p (k i) -> p k i", k=NK))

            def p2_v(t):
                # v (all heads, token-major) for tile t -> ccq v section
                b, sc = t // 4, t % 4
                ps = vps.tile([128, D], F32, tag="v")
                for k in range(NK):
                    nc.tensor.matmul(ps, xnT[:, k, t * 128:(t + 1) * 128],
                                     wv_sb[:, k, :],
                                     start=(k == 0), stop=(k == NK - 1))
                stv = stg.tile([128, H, Dh], BF16, tag="stv")
                if has_bv:
                    nc.vector.tensor_add(
                        stv.rearrange("p h d -> p (h d)"), ps, bv_bc)
                else:
                    nc.scalar.copy(out=stv.rearrange("p h d -> p (h d)"), in_=ps)
                nc.sync.dma_start(
                    out=ccq_in[b][:, :, SL + sc * Dh:SL + (sc + 1) * Dh].rearrange(
                        "w p d -> p w d"),
                    in_=stv)

            def p2_qk(b):
                # q,k head-major for batch b -> ccq qk section (one ship DMA)
                stq = stg.tile([128, H, SL], BF16, tag="stq")
                for h in range(H):
                    ps = qkps.tile([128, SL], F32, tag="qk")
                    for k in range(NK):
                        nc.tensor.matmul(ps, wqk_sb[:, k, h * 128:(h + 1) * 128],
                                         xrT[:, k, b * SL:(b + 1) * SL],
                                         start=(k == 0), stop=(k == NK - 1))
                    if has_bqk:
                        nc.vector.tensor_scalar_add(stq[:, h, :], ps,
                                                    bqk_sb[:, h:h + 1])
                    else:
                        nc.scalar.copy(out=stq[:, h, :], in_=ps)
                nc.sync.dma_start(
                    out=ccq_in[b][:, :, 0:SL].rearrange("w p s -> p w s"),
                    in_=stq)

            def qkv_a2a(b):
                if not skip_cc and max_phase >= 2:
                    nc.gpsimd.collective_compute(
                        "AllToAll", ALU.bypass,
                        ins=[ccq_in[b].opt()], outs=[ccq_out[b].opt()],
                        replica_groups=[list(range(W))],
                    )

            for t in range(4):
                p1_tile(t)
            # q/k/v weights: needed right below; emitted after the first
            # tiles' src/rope DMAs so those aren't queued behind them
            nc.sync.dma_start(out=wqk_sb,
                              in_=wqk_t.rearrange("(k p) m -> p k m", p=128))
            nc.sync.dma_start(out=wv_sb,
                              in_=wv_t.rearrange("(k p) m -> p k m", p=128))
            for t in range(4):
                p2_v(t)
            p2_qk(0)
            qkv_a2a(0)
            for t in range(4, NT):
                p1_tile(t)
            for t in range(4, NT):
                p2_v(t)
            p2_qk(1)
            # out_proj weight: consumed by the P5a overlap during attention
            nc.sync.dma_start(out=wo_sb,
                              in_=wo_t.rearrange("(k p) n -> p k n", p=128))

        # ---- P5 resources (shared between attention overlap + tail) ----
        late = top.enter_context(tc.tile_pool(name="late", bufs=1))
        out1 = late.tile([128, NT, D], F32)   # post-attention residual stream
        yT = late.tile([128, NK, TL], F32R)    # LN2 output, D-major
        sb5 = top.enter_context(tc.tile_pool(name="p5", bufs=2))
        small5 = top.enter_context(tc.tile_pool(name="p5s", bufs=4))
        p5ps = top.enter_context(tc.tile_pool(name="p5ps", bufs=2, space="PSUM"))

        mv_all = late.tile([128, NT, 2], F32)  # LN2 mean/var per tile
        rstd_all = late.tile([128, NT], F32)
        nm_all = late.tile([128, NT], F32)
        rsq_tmp = late.tile([128, 3, NT], F32)
        rsq_i = late.tile([128, NT], mybir.dt.int32)

        def ln2_rsqrt(lo, hi):
            # Batched DVE rsqrt (quake seed + 2 Newton steps) for tiles
            # [lo, hi): keeps LN2 off ScalarE so no exp/gelu table switches.
            n = hi - lo
            xe = rsq_tmp[:, 0, lo:hi]
            xh = rsq_tmp[:, 1, lo:hi]
            nc.vector.tensor_scalar_add(xe, mv_all[:, lo:hi, 1], EPS)
            nc.vector.tensor_scalar_mul(xh, xe, -0.5)
            ib = rsq_i[:, lo:hi]
            nc.vector.tensor_scalar(out=ib, in0=xe.bitcast(mybir.dt.int32),
                                    scalar1=1, scalar2=None,
                                    op0=ALU.logical_shift_right)
            # MAGIC - (i >> 1)  ==  (~(i>>1)) + (MAGIC + 1)
            nc.vector.tensor_scalar(out=ib, in0=ib, scalar1=-1, scalar2=None,
                                    op0=ALU.bitwise_xor)
            nc.vector.tensor_scalar(out=ib, in0=ib, scalar1=0x5f3759df + 1,
                                    scalar2=None, op0=ALU.add)
            y = rstd_all[:, lo:hi]
            nc.vector.tensor_copy(y, ib.bitcast(F32))
            t2 = rsq_tmp[:, 2, lo:hi]
            for _ in range(2):
                nc.vector.tensor_mul(t2, y, y)
                nc.vector.tensor_mul(t2, t2, xh)
                nc.vector.tensor_scalar_add(t2, t2, 1.5)
                nc.vector.tensor_mul(y, y, t2)
            nc.vector.tensor_mul(nm_all[:, lo:hi], mv_all[:, lo:hi, 0], y)
            nc.vector.tensor_scalar_mul(nm_all[:, lo:hi], nm_all[:, lo:hi], -1.0)

        def p5a_tile(t):
            # out_proj + residual + LN2 stats (PE/DVE only — safe to overlap
            # attention without touching ScalarE's loaded exp table set)
            b, sc = t // (NT // B), t % (NT // B)
            po = p5ps.tile([128, D], F32, tag="p5")
            for k in range(NK):
                a_sb = sb5.tile([128, 128], BF16, tag="a")
                nc.gpsimd.dma_start(
                    out=a_sb,
                    in_=cc2_out[b][2 * k:2 * k + 2, :,
                                   sc * 128:(sc + 1) * 128].rearrange(
                                       "e d i -> (e d) i"))
                nc.tensor.matmul(po, a_sb, wo_sb[:, k, :],
                                 start=(k == 0), stop=(k == NK - 1))
            s_t = sb5.tile([128, D], F32, tag="s")
            nc.sync.dma_start(out=s_t, in_=src_loc[t * 128:(t + 1) * 128, :])
            o1 = out1[:, t, :]
            nc.vector.tensor_add(o1, po, s_t)
            if has_bo:
                nc.vector.tensor_add(o1, o1, bo_bc)
            stats = small5.tile([128, 6], F32, tag="st")
            nc.vector.bn_stats(out=stats, in_=o1)
            nc.vector.bn_aggr(out=mv_all[:, t, :], in_=stats)

        def p5b_tile(t):
            # LN2 normalize + yT transpose (rstd/nm precomputed on DVE)
            y_t = sb5.tile([128, D], F32, tag="y")
            nc.vector.tensor_scalar(out=y_t, in0=out1[:, t, :],
                                    scalar1=rstd_all[:, t:t + 1],
                                    scalar2=nm_all[:, t:t + 1],
                                    op0=ALU.mult, op1=ALU.add)
            ps = p5ps.tile([128, 512], F32, tag="p5")
            for k in range(NK):
                nc.tensor.transpose(ps[:, k * 128:(k + 1) * 128],
                                    y_t[:, k * 128:(k + 1) * 128], ident)
            nc.scalar.copy(
                out=yT[:, :, t * 128:(t + 1) * 128],
                in_=ps.rearrange("p (k i) -> p k i", k=NK))

        with ExitStack() as actx:
          if max_phase >= 3:
            act = actx.enter_context(tc.tile_pool(name="act", bufs=1))
            # per-batch tiles: batch b's attention must not depend on the
            # other batch's AllToAll (dep tracking is tile-granular)
            qT = [act.tile([Dh, S], BF16, name=f"qT{b}") for b in range(B)]
            kT = [act.tile([Dh, S], BF16, name=f"kT{b}") for b in range(B)]
            vS = [act.tile([128, NS, 65], BF16, name=f"vS{b}") for b in range(B)]
            attnT = [act.tile([Dh, S], BF16, name=f"attnT{b}") for b in range(B)]
            for b in range(B):
                nc.vector.memset(vS[b][:, :, 64:65], 1.0)

            # ---- assembly: head c = my rank's block. b0's assembly is
            # emitted BEFORE the b1 AllToAll: deps collapse to per-engine
            # counters, so anything emitted after the second collective
            # waits for both. ----
            def assemble(b):
                eng = nc.sync if b == 0 else nc.gpsimd
                eng.dma_start(
                    out=qT[b].rearrange("p (j s) -> p j s", j=W),
                    in_=ccq_out[b][:, 0:Dh, 0:SL].rearrange("j p s -> p j s"))
                eng.dma_start(
                    out=kT[b].rearrange("p (j s) -> p j s", j=W),
                    in_=ccq_out[b][:, Dh:2 * Dh, 0:SL].rearrange(
                        "j p s -> p j s"))
                for j in range(W):
                    eng.dma_start(
                        out=vS[b][:, j * 4:(j + 1) * 4, 0:64],
                        in_=ccq_out[b][j, :, SL:SL + 4 * Dh]
                            .rearrange("p (sc d) -> p sc d", d=Dh))

            assemble(0)
            qkv_a2a(1)
            # FFN weights on the SWDGE (gpsimd) queues: keeps them off the
            # hardware-DMA count barriers that gate attention's first matmul
            nc.gpsimd.dma_start(out=w1_sb,
                                in_=w1_t.rearrange("(k p) n -> p k n", p=128))
            nc.gpsimd.dma_start(out=w2_sb,
                                in_=w2_t.rearrange("(m p) n -> p m n", p=128))
            nc.gpsimd.dma_start(out=b1_sb, in_=b1p.rearrange("(m p) -> p m", p=128))

            # ============ P4: causal attention (software-pipelined) ============
            if max_phase >= 4:
              with ExitStack() as ctx:
                  expp = ctx.enter_context(tc.tile_pool(name="p4e", bufs=6))
                  nrm = ctx.enter_context(tc.tile_pool(name="p4n", bufs=3))
                  scps = ctx.enter_context(tc.tile_pool(name="p4s", bufs=2, space="PSUM"))
                  atps = ctx.enter_context(tc.tile_pool(name="p4a", bufs=2, space="PSUM"))
                  # flat job list: (b, qb, pair)
                  jobs = [(b, qb, p)
                          for b in range(B) for qb in range(8)
                          for p in range(2 * (qb + 1))]
                  sc_ps = {}
                  pa_cur = {}

                  def emit_sc(job):
                      b, qb, p = job
                      q_rhs = qT[b][:, qb * 512:(qb + 1) * 512]
                      ps = scps.tile([128, 1024], F32, tag="sc", name="sc_ps_t")
                      for i in range(2):
                          kt = p * 2 + i
                          nc.tensor.matmul(ps[:, i * 512:(i + 1) * 512],
                                           kT[b][:, kt * 128:(kt + 1) * 128],
                                           q_rhs, start=True, stop=True)
                      sc_ps[job] = ps

                  def emit_pv(job):
                      b, qb, p = job
                      nkt = 4 * (qb + 1)
                      ps = sc_ps.pop(job)
                      if p == 0:
                          pa_cur[(b, qb)] = atps.tile([65, 512], F32, tag="pa",
                                                      name="pa_t")
                      pa = pa_cur[(b, qb)]
                      ex = expp.tile([128, 1024], BF16, tag="ex", name="ex_t")
                      nc.scalar.activation(out=ex, in_=ps, func=AF.Exp, scale=SCALE)
                      for i in range(2):
                          kt = p * 2 + i
                          jm = kt - (nkt - 4)
                          if jm >= 0:
                              nc.vector.tensor_mul(ex[:, i * 512:(i + 1) * 512],
                                                   ex[:, i * 512:(i + 1) * 512],
                                                   masks[:, jm, :])
                          nc.tensor.matmul(pa, vS[b][:, kt, :],
                                           ex[:, i * 512:(i + 1) * 512],
                                           start=(kt == 0), stop=(kt == nkt - 1))
                      if p == 2 * (qb + 1) - 1:
                          # normalization tail for this (b, qb)
                          pa = pa_cur.pop((b, qb))
                          pa_sb = nrm.tile([65, 512], F32, tag="pasb")
                          nc.vector.tensor_copy(pa_sb, pa)
                          # reciprocal in place on partition 64, then replicate
                          # across 64 partitions on the PE (no partition-move
                          # DMA; Pool engine is busy with the next AllToAll)
                          nc.vector.reciprocal(pa_sb[64:65, :], pa_sb[64:65, :])
                          rcp_ps = p5ps.tile([Dh, 512], F32, tag="p5")
                          nc.tensor.matmul(rcp_ps, ones_row, pa_sb[64:65, :],
                                           start=True, stop=True)
                          nc.vector.tensor_mul(
                              attnT[b][:, qb * 512:(qb + 1) * 512],
                              pa_sb[0:64, :], rcp_ps)

                  # P5 for b0 token tiles overlaps b1's attention (deps via
                  # cc2_out[0], ready once the b0 AllToAll lands)
                  p5_overlap = {83: 0, 95: 1, 107: 2, 119: 3} if max_phase >= 6 \
                      else {}
                  emit_sc(jobs[0])
                  for idx, job in enumerate(jobs):
                      if idx + 1 < len(jobs):
                          emit_sc(jobs[idx + 1])
                      emit_pv(job)
                      # ship + exchange each batch as soon as it completes
                      b, qb, p = job
                      if qb == 7 and p == 2 * (qb + 1) - 1:
                          nc.sync.dma_start(
                              out=cc2_in[b].rearrange("j d i -> d j i"),
                              in_=attnT[b].rearrange("d (j i) -> d j i", j=W))
                          if not skip_cc and max_phase >= 5:
                              nc.gpsimd.collective_compute(
                                  "AllToAll", ALU.bypass,
                                  ins=[cc2_in[b].opt()], outs=[cc2_out[b].opt()],
                                  replica_groups=[list(range(W))],
                              )
                      if idx == 70:
                          assemble(1)
                      if idx in p5_overlap:
                          p5a_tile(p5_overlap[idx])
                      if idx == 125 and max_phase >= 7:
                          ln2_rsqrt(0, 4)
                      if idx in (131, 134, 137, 140) and max_phase >= 7:
                          p5b_tile(idx // 3 - 43)

        # ========== tail: P6(th0) -> P5(b1) -> P6(th1) ==========
        # P6 th0 depends only on b0's yT (done during attention), so it runs
        # on PE while the b1 AllToAll completes in the background.
        if max_phase >= 7:
          with ExitStack() as ctx:
              sb = ctx.enter_context(tc.tile_pool(name="p6", bufs=3))
              hps = ctx.enter_context(tc.tile_pool(name="p6h", bufs=2, space="PSUM"))
              o2ps = ctx.enter_context(tc.tile_pool(name="p6o", bufs=1, space="PSUM"))

              def p6_half(th):
                  po2 = [o2ps.tile([128, D], F32, tag=f"po2_{tq}", name=f"po2_{tq}")
                         for tq in range(4)]
                  for m in range(NF):
                      ph = hps.tile([128, 512], F32, tag="ph")
                      for k in range(NK):
                          nc.tensor.matmul(ph, w1_sb[:, k, m * 128:(m + 1) * 128],
                                           yT[:, k, th * 512:(th + 1) * 512],
                                           start=(k == 0), stop=(k == NK - 1))
                      hT = sb.tile([128, 512], F32R, tag="hT")
                      nc.scalar.activation(out=hT, in_=ph,
                                           func=_GELU_OVERRIDE or AF.Gelu,
                                           bias=b1_sb[:, m:m + 1])
                      for tq in range(4):
                          nc.tensor.matmul(po2[tq], hT[:, tq * 128:(tq + 1) * 128],
                                           w2_sb[:, m, :],
                                           start=(m == 0), stop=(m == NF - 1))
                  for tq in range(4):
                      t = th * 4 + tq
                      fin = sb.tile([128, D], F32, tag="fin")
                      nc.vector.tensor_add(fin, po2[tq], out1[:, t, :])
                      if has_b2:
                          nc.vector.tensor_add(fin, fin, b2_bc)
                      nc.sync.dma_start(out=out_loc[t * 128:(t + 1) * 128, :], in_=fin)

              p6_half(0)
              for t in range(4, NT):
                  p5a_tile(t)
              ln2_rsqrt(4, NT)
              for t in range(4, NT):
                  p5b_tile(t)
              p6_half(1)

        if max_phase < 7:
            with tc.tile_pool(name="dummy", bufs=1) as dp:
                dt_ = dp.tile([128, D], F32)
                nc.vector.memset(dt_, 0.0)
                for i in range(TL // 128):
                    nc.sync.dma_start(out=out_loc[i * 128:(i + 1) * 128, :], in_=dt_)
    nc.compile()
    return nc


def _prep(inputs):
    src = np.asarray(inputs["src"], np.float32)
    cos = np.asarray(inputs["rotary_cos"], np.float32).reshape(S, Dh)
    sin = np.asarray(inputs["rotary_sin"], np.float32).reshape(S, Dh)
    ipw = np.asarray(inputs["in_proj_w"], np.float32)
    ipb = np.asarray(inputs["in_proj_b"], np.float32)
    opw = np.asarray(inputs["out_proj_w"], np.float32)
    opb = np.asarray(inputs["out_proj_b"], np.float32)
    w1 = np.asarray(inputs["w1"], np.float32)
    b1 = np.asarray(inputs["b1"], np.float32)
    w2 = np.asarray(inputs["w2"], np.float32)
    b2 = np.asarray(inputs["b2"], np.float32)
    ln1_w = np.asarray(inputs["ln1_w"], np.float32)
    ln1_b = np.asarray(inputs["ln1_b"], np.float32)
    ln2_w = np.asarray(inputs["ln2_w"], np.float32)
    ln2_b = np.asarray(inputs["ln2_b"], np.float32)

    cos_full = np.tile(cos, (1, H))            # [S, D]
    sin_full = np.tile(sin, (1, H))
    d = np.arange(D)
    jj = d % Dh
    hb = d - jj
    src2 = np.where(jj < 32, hb + 2 * jj + 1, hb + 2 * (jj - 32))
    sign = np.where(jj < 32, -1.0, 1.0).astype(np.float32)
    cosw_full = ln1_w[None, :] * cos_full
    rotw_full = (sign[None, :] * ln1_w[src2][None, :]) * sin_full
    ropeb_full = (ln1_b[None, :] * cos_full
                  + (sign[None, :] * ln1_b[src2][None, :]) * sin_full)

    wq, wk, wv = ipw[0:D], ipw[D:2 * D], ipw[2 * D:3 * D]
    bq, bk, bvv = ipb[0:D], ipb[D:2 * D], ipb[2 * D:3 * D]
    # q,k packed h-major: [wq_h.T | wk_h.T] per head
    wqk_cols = []
    for h in range(H):
        wqk_cols.append(wq[h * Dh:(h + 1) * Dh].T)
        wqk_cols.append(wk[h * Dh:(h + 1) * Dh].T)
    wqk_t = np.ascontiguousarray(np.concatenate(wqk_cols, axis=1))  # [D, 1024]
    bqk_pack = np.zeros((128, H), np.float32)
    for h in range(H):
        bqk_pack[0:Dh, h] = bq[h * Dh:(h + 1) * Dh]
        bqk_pack[Dh:2 * Dh, h] = bk[h * Dh:(h + 1) * Dh]
    wv_t = np.ascontiguousarray(ln1_w[:, None] * wv.T, np.float32)  # [D, 512]
    bv_all = np.ascontiguousarray(ln1_b @ wv.T + bvv, np.float32)
    w1_t = np.ascontiguousarray(ln2_w[:, None] * w1.T, np.float32)   # [D, F]
    b1p = np.ascontiguousarray(ln2_b @ w1.T + b1, np.float32)
    wo_t = np.ascontiguousarray(opw.T)

    flags = (
        bool(np.any(ropeb_full)), bool(np.any(bq) or np.any(bk)),
        bool(np.any(bvv) or np.any(ln1_b)), bool(np.any(opb)), bool(np.any(b2)),
    )

    shared = {
        "wqk_t": wqk_t.astype(ml_dtypes.bfloat16),
        "wv_t": wv_t.astype(ml_dtypes.bfloat16),
        "bqk": bqk_pack,
        "bv": bv_all,
        "wo_t": wo_t.astype(ml_dtypes.bfloat16),
        "bo": opb,
        "w1_t": w1_t,
        "b1p": b1p,
        "w2_t": np.ascontiguousarray(w2.T),
        "b2": b2,
    }
    in_maps = []
    for c in range(W):
        m = dict(shared)
        m["src_loc"] = np.ascontiguousarray(
            src[SL * c:SL * (c + 1)].transpose(1, 0, 2).reshape(TL, D))
        m["cosw"] = np.ascontiguousarray(cosw_full[SL * c:SL * (c + 1)])
        m["rotw"] = np.ascontiguousarray(rotw_full[SL * c:SL * (c + 1)])
        if flags[0]:
            m["ropeb"] = np.ascontiguousarray(ropeb_full[SL * c:SL * (c + 1)])
        in_maps.append(m)
    return in_maps, flags


def _get_nc(flags):
    if flags not in _NC_CACHE:
        _NC_CACHE[flags] = _build_nc(flags)
    return _NC_CACHE[flags]


def kernel(**inputs):
    in_maps, flags = _prep(inputs)
    nc = _get_nc(flags)
    res = run_bass_kernel_spmd(nc, in_maps, core_ids=list(range(W)))
    out = np.empty((S, B, D), np.float32)
    for c in range(W):
        ol = res.results[c]["out_loc"].reshape(B, SL, D)
        out[SL * c:SL * (c + 1)] = ol.transpose(1, 0, 2)
    return out



# revision 20
# speedup vs baseline: 1.1156x; 1.1156x over previous
"""Trainium2 Bass kernel: transformer encoder layer (S=4096,B=2,D=512,H=8,F=2048),
causal attention + RoPE, distributed over 8 NeuronCores.

Sharding (SPMD: one program, per-core data):
  - LN1+RoPE: sequence-parallel (core c owns s in [512c, 512(c+1)), both batches)
  - QKV projections: token-parallel (each core projects q,k,v of ALL heads for
    its own tokens, emitting q/k head-major and v token-major directly)
  - AllToAll(qkv)             [384KB/rank] -> core c gets head c for all tokens
  - causal attention: head-parallel (core c owns head c, full S, both b)
  - AllToAll(attn_head^T)     [512KB/rank, per batch] -> all heads, own tokens
  - out_proj + residual + LN2 + FFN: token-parallel (core c owns its s-slice)
LayerNorm affine params are folded into downstream weights host-side.
Softmax denominators come free from a ones-column appended to V.
All weights preloaded to SBUF up front so their DMA overlaps early compute.
"""
import numpy as np
import ml_dtypes
from contextlib import ExitStack

import concourse.bass as bass
import concourse.tile as tile
from concourse import bacc, mybir
from concourse.bass_utils import run_bass_kernel_spmd
from concourse.masks import make_identity

F32 = mybir.dt.float32
F32R = mybir.dt.float32r
BF16 = mybir.dt.bfloat16
FP8 = mybir.dt.float8e4
AF = mybir.ActivationFunctionType
ALU = mybir.AluOpType
DR = mybir.MatmulPerfMode.DoubleRow
LN64 = float(np.log(64.0))  # exp bias: scales probs+denom by 64 (cancels in
                            # normalization) to keep fp8e4 values well-normal

S, B, D, H, Dh, F = 4096, 2, 512, 8, 64, 2048
W = 8                    # cores
SL = S // W              # 512 s-positions per core
TL = SL * B              # 1024 local tokens
EPS = 1e-5
SCALE = 1.0 / float(np.sqrt(Dh))  # 0.125

NT = TL // 128           # 8 local token tiles
NK = D // 128            # 4 contraction chunks over D
NF = F // 128            # 16 chunks over F
NS = S // 128            # 32 key tiles per batch
QKC = 2 * Dh * H         # 1024 packed q,k output columns (h-major)
CCW = QKC + NT * Dh      # 1536 columns per A2A block

_NC_CACHE = {}
_GELU_OVERRIDE = None  # set to AF.Identity in sim tests (CoreSim lacks Gelu)


def _layer_norm_stats(nc, pool, x_t, eps_sb):
    """Returns (rstd [128,1], negmean_rstd [128,1]) for rows of x_t."""
    stats = pool.tile([128, 6], F32, tag="st")
    nc.vector.bn_stats(out=stats, in_=x_t)
    mv = pool.tile([128, 2], F32, tag="mv")
    nc.vector.bn_aggr(out=mv, in_=stats)
    sd = pool.tile([128, 1], F32, tag="sd")
    nc.scalar.activation(out=sd, in_=mv[:, 1:2], func=AF.Sqrt, bias=eps_sb)
    rstd = pool.tile([128, 1], F32, tag="rs")
    nc.vector.reciprocal(out=rstd, in_=sd)
    nm = pool.tile([128, 1], F32, tag="nm")
    nc.vector.tensor_mul(nm, mv[:, 0:1], rstd)
    nc.vector.tensor_scalar_mul(nm, nm, -1.0)
    return rstd, nm


def _build_nc(flags, n_reps=1):
    """flags = (has_ropeb, has_bqk, has_bv, has_bo, has_b2)

    n_reps > 1 builds a timing variant with the body unrolled n_reps times
    (same I/O, idempotent) so device time can be read off the slope.
    """
    import os as _os
    has_ropeb, has_bqk, has_bv, has_bo, has_b2 = flags
    skip_cc = bool(int(_os.environ.get("K_SKIP_CC", "0")))
    max_phase = int(_os.environ.get("K_MAX_PHASE", "7"))
    nc = bacc.Bacc("TRN2", target_bir_lowering=False, debug=False, num_devices=W)

    # ---- I/O ----
    src_loc = nc.dram_tensor("src_loc", [TL, D], F32, kind="ExternalInput")
    cosw = nc.dram_tensor("cosw", [SL, D], BF16, kind="ExternalInput")
    rotw = nc.dram_tensor("rotw", [SL, D], BF16, kind="ExternalInput")
    ropeb = nc.dram_tensor("ropeb", [SL, D], F32, kind="ExternalInput") if has_ropeb else None
    wqk_t = nc.dram_tensor("wqk_t", [D, QKC], BF16, kind="ExternalInput")
    wv_t = nc.dram_tensor("wv_t", [D, D], BF16, kind="ExternalInput")
    bqk = nc.dram_tensor("bqk", [128, H], F32, kind="ExternalInput")
    bv = nc.dram_tensor("bv", [D], F32, kind="ExternalInput")
    wo_t = nc.dram_tensor("wo_t", [D, D], BF16, kind="ExternalInput")
    bo = nc.dram_tensor("bo", [D], F32, kind="ExternalInput")
    w1_t = nc.dram_tensor("w1_t", [D, F], F32R, kind="ExternalInput")
    b1p = nc.dram_tensor("b1p", [F], F32, kind="ExternalInput")
    w2_t = nc.dram_tensor("w2_t", [F, D], F32R, kind="ExternalInput")
    b2 = nc.dram_tensor("b2", [D], F32, kind="ExternalInput")
    out_loc = nc.dram_tensor("out_loc", [TL, D], F32, kind="ExternalOutput")

    with tile.TileContext(nc) as tc:
      for _rep in range(n_reps):
       with ExitStack() as top:
        dram = top.enter_context(tc.tile_pool(name="dram", bufs=1, space="DRAM"))
        consts = top.enter_context(tc.tile_pool(name="consts", bufs=1))

        # ---------- constants + all weights up front ----------
        ident = consts.tile([128, 128], F32)
        make_identity(nc, ident)
        # causal diag masks: masks[:, j, q] = 1.0 if q >= k + j*128 else 0.0
        masks_bf = consts.tile([128, 4, 512], BF16)
        for j in range(4):
            nc.gpsimd.memset(masks_bf[:, j, :], 1.0)
            nc.gpsimd.affine_select(
                out=masks_bf[:, j, :], in_=masks_bf[:, j, :],
                compare_op=ALU.is_ge, fill=0.0,
                base=-j * 128, channel_multiplier=-1, pattern=[[1, 512]],
            )
        masks = consts.tile([128, 4, 512], FP8)
        nc.vector.tensor_copy(masks, masks_bf)
        eps_sb = consts.tile([128, 1], F32)
        nc.vector.memset(eps_sb, EPS)
        ln64_sb = consts.tile([128, 1], F32)
        nc.vector.memset(ln64_sb, LN64)
        ones_col = consts.tile([65, Dh], F32)
        nc.vector.memset(ones_col, 1.0)
        ones_row = ones_col[64:65, :]   # base partition 64, matches pa_sb row
        # weight SBUF tiles declared here, DMAs emitted later (so the P1/P2'
        # token loads aren't queued behind 9MB of weight traffic)
        wqk_sb = consts.tile([128, NK, QKC], BF16)
        wv_sb = consts.tile([128, NK, D], BF16)
        wo_sb = consts.tile([128, NK, D], BF16)
        w1_sb = consts.tile([128, NK, F], F32R)
        w2_sb = consts.tile([128, NF, D], F32R)
        b1_sb = consts.tile([128, NF], F32)
        bqk_sb = consts.tile([128, H], F32)
        if has_bqk:
            nc.sync.dma_start(out=bqk_sb, in_=bqk)
        bv_bc = consts.tile([128, D], F32)
        if has_bv:
            bv_row = consts.tile([1, D], F32)
            nc.sync.dma_start(out=bv_row, in_=bv[None, :])
            nc.gpsimd.partition_broadcast(bv_bc, bv_row)
        bo_bc = consts.tile([128, D], F32)
        if has_bo:
            bo_row = consts.tile([1, D], F32)
            nc.sync.dma_start(out=bo_row, in_=bo[None, :])
            nc.gpsimd.partition_broadcast(bo_bc, bo_row)
        b2_bc = consts.tile([128, D], F32)
        if has_b2:
            b2_row = consts.tile([1, D], F32)
            nc.sync.dma_start(out=b2_row, in_=b2[None, :])
            nc.gpsimd.partition_broadcast(b2_bc, b2_row)

        # collective buffers (qkv exchange split per batch so b0's AllToAll
        # overlaps b1's LN/RoPE/projection compute). Byte tile, flat per dest
        # rank: qk block [p=128][s=512] bf16 (viewed via bitcast), then v
        # block [sc=4][p=128][d=64] fp8 for the DoubleRow PV matmul.
        QKB = 2 * Dh * SL * 2            # 131072 qk bytes per rank
        VB = 128 * 4 * Dh                # 32768 v bytes per rank
        ccq_in = [dram.tile([W, QKB + VB], FP8, name=f"ccq_in_{b}")
                  for b in range(B)]
        ccq_out = [dram.tile([W, QKB + VB], FP8, name=f"ccq_out_{b}")
                   for b in range(B)]
        cc2_in = [dram.tile([W, Dh, SL], BF16, name=f"cc2_in_{b}") for b in range(B)]
        cc2_out = [dram.tile([W, Dh, SL], BF16, name=f"cc2_out_{b}") for b in range(B)]

        # ============ P1: LN1 + RoPE + transpose (seq-parallel) ============
        # ============ P2': QKV for own tokens, all heads ============
        with ExitStack() as ctx:
            xt_pool = ctx.enter_context(tc.tile_pool(name="xT", bufs=1))
            # D-major LN1/RoPE outputs (live through P2' only)
            xrT = xt_pool.tile([128, NK, TL], BF16)
            xnT = xt_pool.tile([128, NK, TL], BF16)
            sb = ctx.enter_context(tc.tile_pool(name="p1", bufs=3))
            small = ctx.enter_context(tc.tile_pool(name="p1s", bufs=4))
            ropet = ctx.enter_context(tc.tile_pool(name="p1r", bufs=1))
            # rope tables loaded per-sc chunk inside p1_tile (keeps the DMA
            # queue prioritized for the b0 critical path)
            cosw_all = ropet.tile([128, 4, D], BF16)
            rotw_all = ropet.tile([128, 4, D], BF16)
            trps = ctx.enter_context(tc.tile_pool(name="p1ps", bufs=2, space="PSUM"))
            qkps = ctx.enter_context(tc.tile_pool(name="p2qk", bufs=2, space="PSUM"))
            vps = ctx.enter_context(tc.tile_pool(name="p2v", bufs=2, space="PSUM"))
            stg = ctx.enter_context(tc.tile_pool(name="p2stg", bufs=3))

            def p1_tile(t):
                s_t = sb.tile([128, D], F32, tag="s")
                nc.sync.dma_start(out=s_t, in_=src_loc[t * 128:(t + 1) * 128, :])
                if t < 4:
                    nc.sync.dma_start(
                        out=cosw_all[:, t, :],
                        in_=cosw.rearrange("(s p) d -> p s d", p=128)[:, t, :])
                    nc.sync.dma_start(
                        out=rotw_all[:, t, :],
                        in_=rotw.rearrange("(s p) d -> p s d", p=128)[:, t, :])
                rstd, nm = _layer_norm_stats(nc, small, s_t, eps_sb)
                xn_t = sb.tile([128, D], F32, tag="xn")
                nc.vector.tensor_scalar(
                    out=xn_t, in0=s_t, scalar1=rstd, scalar2=nm,
                    op0=ALU.mult, op1=ALU.add,
                )
                # RoPE (ln1 affine folded into cosw/rotw/ropeb host-side)
                sc = t % (SL // 128)
                cosw_t = cosw_all[:, sc, :]
                rotw_t = rotw_all[:, sc, :]
                xr_t = sb.tile([128, D], F32, tag="xr")
                rt = sb.tile([128, D], F32, tag="rt")
                xnv = xn_t.rearrange("p (h i two) -> p h i two", h=H, two=2)
                rtv = rt.rearrange("p (h d) -> p h d", h=H)
                rwv = rotw_t.rearrange("p (h d) -> p h d", h=H)
                # rt[:, :, :32] = xn[:, :, 1::2] * rotw[:, :, :32]
                nc.vector.tensor_mul(rtv[:, :, 0:32], xnv[:, :, :, 1], rwv[:, :, 0:32])
                # rt[:, :, 32:] = xn[:, :, 0::2] * rotw[:, :, 32:]
                nc.vector.tensor_mul(rtv[:, :, 32:64], xnv[:, :, :, 0], rwv[:, :, 32:64])
                nc.vector.tensor_mul(xr_t, xn_t, cosw_t)
                nc.vector.tensor_add(xr_t, xr_t, rt)
                if has_ropeb:
                    rb_t = sb.tile([128, D], F32, tag="rb")
                    nc.sync.dma_start(out=rb_t, in_=ropeb[sc * 128:(sc + 1) * 128, :])
                    nc.vector.tensor_add(xr_t, xr_t, rb_t)
                # transpose both to D-major; xr -> DVE copy, xn -> ScalarE copy
                ps_r = trps.tile([128, 512], F32, tag="trr")
                ps_n = trps.tile([128, 512], F32, tag="trn")
                for k in range(NK):
                    nc.tensor.transpose(ps_r[:, k * 128:(k + 1) * 128],
                                        xr_t[:, k * 128:(k + 1) * 128], ident)
                    nc.tensor.transpose(ps_n[:, k * 128:(k + 1) * 128],
                                        xn_t[:, k * 128:(k + 1) * 128], ident)
                nc.vector.tensor_copy(
                    xrT[:, :, t * 128:(t + 1) * 128],
                    ps_r.rearrange("p (k i) -> p k i", k=NK))
                nc.scalar.copy(
                    out=xnT[:, :, t * 128:(t + 1) * 128],
                    in_=ps_n.rearrange("p (k i) -> p k i", k=NK))

            def p2_v(t):
                # v (all heads, token-major) for tile t -> ccq v section
                b, sc = t // 4, t % 4
                ps = vps.tile([128, D], F32, tag="v")
                for k in range(NK):
                    nc.tensor.matmul(ps, xnT[:, k, t * 128:(t + 1) * 128],
                                     wv_sb[:, k, :],
                                     start=(k == 0), stop=(k == NK - 1))
                stv = stg.tile([128, H, Dh], FP8, tag="stv")
                if has_bv:
                    nc.vector.tensor_add(
                        stv.rearrange("p h d -> p (h d)"), ps, bv_bc)
                else:
                    nc.vector.tensor_copy(stv.rearrange("p h d -> p (h d)"), ps)
                nc.sync.dma_start(
                    out=ccq_in[b][:, QKB:].rearrange(
                        "w (sc p d) -> sc p w d", sc=4, p=128)[sc],
                    in_=stv)

            def p2_qk(b):
                # q,k head-major for batch b -> ccq qk section (one ship DMA)
                stq = stg.tile([128, H, SL], BF16, tag="stq")
                for h in range(H):
                    ps = qkps.tile([128, SL], F32, tag="qk")
                    for k in range(NK):
                        nc.tensor.matmul(ps, wqk_sb[:, k, h * 128:(h + 1) * 128],
                                         xrT[:, k, b * SL:(b + 1) * SL],
                                         start=(k == 0), stop=(k == NK - 1))
                    if has_bqk:
                        nc.vector.tensor_scalar_add(stq[:, h, :], ps,
                                                    bqk_sb[:, h:h + 1])
                    else:
                        nc.vector.tensor_copy(stq[:, h, :], ps)
                nc.sync.dma_start(
                    out=ccq_in[b][:, 0:QKB].bitcast(BF16).rearrange(
                        "w (p s) -> p w s", p=128),
                    in_=stq)

            def qkv_a2a(b):
                if not skip_cc and max_phase >= 2:
                    nc.gpsimd.collective_compute(
                        "AllToAll", ALU.bypass,
                        ins=[ccq_in[b].opt()], outs=[ccq_out[b].opt()],
                        replica_groups=[list(range(W))],
                    )

            p1_tile(0)
            # q/k/v weights: needed right below; emitted after tile 0's
            # src/rope DMAs so those aren't queued behind them
            nc.sync.dma_start(out=wqk_sb,
                              in_=wqk_t.rearrange("(k p) m -> p k m", p=128))
            nc.sync.dma_start(out=wv_sb,
                              in_=wv_t.rearrange("(k p) m -> p k m", p=128))
            for t in range(1, 4):
                p1_tile(t)
            for t in range(4):
                p2_v(t)
            p2_qk(0)
            qkv_a2a(0)
            for t in range(4, NT):
                p1_tile(t)
            for t in range(4, NT):
                p2_v(t)
            p2_qk(1)
            # out_proj weight: consumed by the P5a overlap during attention
            nc.sync.dma_start(out=wo_sb,
                              in_=wo_t.rearrange("(k p) n -> p k n", p=128))

        # ---- P5 resources (shared between attention overlap + tail) ----
        late = top.enter_context(tc.tile_pool(name="late", bufs=1))
        out1 = late.tile([128, NT, D], F32)   # post-attention residual stream
        yT = late.tile([128, NK, TL], F32R)    # LN2 output, D-major
        sb5 = top.enter_context(tc.tile_pool(name="p5", bufs=2))
        small5 = top.enter_context(tc.tile_pool(name="p5s", bufs=4))
        p5ps = top.enter_context(tc.tile_pool(name="p5ps", bufs=2, space="PSUM"))

        mv_all = late.tile([128, NT, 2], F32)  # LN2 mean/var per tile
        rstd_all = late.tile([128, NT], F32)
        nm_all = late.tile([128, NT], F32)
        rsq_tmp = late.tile([128, 3, NT], F32)
        rsq_i = late.tile([128, NT], mybir.dt.int32)

        def ln2_rsqrt(lo, hi):
            # Batched DVE rsqrt (quake seed + 2 Newton steps) for tiles
            # [lo, hi): keeps LN2 off ScalarE so no exp/gelu table switches.
            n = hi - lo
            xe = rsq_tmp[:, 0, lo:hi]
            xh = rsq_tmp[:, 1, lo:hi]
            nc.vector.tensor_scalar_add(xe, mv_all[:, lo:hi, 1], EPS)
            nc.vector.tensor_scalar_mul(xh, xe, -0.5)
            ib = rsq_i[:, lo:hi]
            nc.vector.tensor_scalar(out=ib, in0=xe.bitcast(mybir.dt.int32),
                                    scalar1=1, scalar2=None,
                                    op0=ALU.logical_shift_right)
            # MAGIC - (i >> 1)  ==  (~(i>>1)) + (MAGIC + 1)
            nc.vector.tensor_scalar(out=ib, in0=ib, scalar1=-1, scalar2=None,
                                    op0=ALU.bitwise_xor)
            nc.vector.tensor_scalar(out=ib, in0=ib, scalar1=0x5f3759df + 1,
                                    scalar2=None, op0=ALU.add)
            y = rstd_all[:, lo:hi]
            nc.vector.tensor_copy(y, ib.bitcast(F32))
            t2 = rsq_tmp[:, 2, lo:hi]
            for _ in range(2):
                nc.vector.tensor_mul(t2, y, y)
                nc.vector.tensor_mul(t2, t2, xh)
                nc.vector.tensor_scalar_add(t2, t2, 1.5)
                nc.vector.tensor_mul(y, y, t2)
            nc.vector.tensor_mul(nm_all[:, lo:hi], mv_all[:, lo:hi, 0], y)
            nc.vector.tensor_scalar_mul(nm_all[:, lo:hi], nm_all[:, lo:hi], -1.0)

        def p5a_tile(t):
            # out_proj + residual + LN2 stats (PE/DVE only — safe to overlap
            # attention without touching ScalarE's loaded exp table set)
            b, sc = t // (NT // B), t % (NT // B)
            po = p5ps.tile([128, D], F32, tag="p5")
            a_sb = sb5.tile([128, NK, 128], BF16, tag="a")
            nc.gpsimd.dma_start(
                out=a_sb,
                in_=cc2_out[b][:, :, sc * 128:(sc + 1) * 128].rearrange(
                    "(k e) d i -> (e d) k i", k=NK))
            for k in range(NK):
                nc.tensor.matmul(po, a_sb[:, k, :], wo_sb[:, k, :],
                                 start=(k == 0), stop=(k == NK - 1))
            s_t = sb5.tile([128, D], F32, tag="s")
            nc.sync.dma_start(out=s_t, in_=src_loc[t * 128:(t + 1) * 128, :])
            o1 = out1[:, t, :]
            nc.vector.tensor_add(o1, po, s_t)
            if has_bo:
                nc.vector.tensor_add(o1, o1, bo_bc)
            stats = small5.tile([128, 6], F32, tag="st")
            nc.vector.bn_stats(out=stats, in_=o1)
            nc.vector.bn_aggr(out=mv_all[:, t, :], in_=stats)

        def p5b_tile(t):
            # LN2 normalize + yT transpose (rstd/nm precomputed on DVE)
            y_t = sb5.tile([128, D], F32, tag="y")
            nc.vector.tensor_scalar(out=y_t, in0=out1[:, t, :],
                                    scalar1=rstd_all[:, t:t + 1],
                                    scalar2=nm_all[:, t:t + 1],
                                    op0=ALU.mult, op1=ALU.add)
            ps = p5ps.tile([128, 512], F32, tag="p5")
            for k in range(NK):
                nc.tensor.transpose(ps[:, k * 128:(k + 1) * 128],
                                    y_t[:, k * 128:(k + 1) * 128], ident)
            # DVE, not ScalarE: the overlapped p5b tiles run while attention
            # saturates ScalarE with exp
            nc.vector.tensor_copy(
                yT[:, :, t * 128:(t + 1) * 128],
                ps.rearrange("p (k i) -> p k i", k=NK))

        with ExitStack() as actx:
          if max_phase >= 3:
            act = actx.enter_context(tc.tile_pool(name="act", bufs=1))
            # per-batch tiles: batch b's attention must not depend on the
            # other batch's AllToAll (dep tracking is tile-granular).
            qT = [act.tile([Dh, S], BF16, name=f"qT{b}") for b in range(B)]
            kT = [act.tile([Dh, S], BF16, name=f"kT{b}") for b in range(B)]
            # vS row stride 80 (not 65): DoubleRow needs the k-tile step to be
            # a multiple of 16 bytes. col 64 = ones (softmax denominator).
            vS = [act.tile([128, NS, 80], FP8, name=f"vS{b}") for b in range(B)]
            attnT = [act.tile([Dh, S], BF16, name=f"attnT{b}") for b in range(B)]
            for b in range(B):
                nc.vector.memset(vS[b][:, :, 64:65], 1.0)

            # ---- assembly: head c = my rank's block. b0's assembly is
            # emitted BEFORE the b1 AllToAll: deps collapse to per-engine
            # counters, so anything emitted after the second collective
            # waits for both. ----
            def assemble(b):
                eng = nc.sync if b == 0 else nc.gpsimd
                qkv = ccq_out[b][:, 0:QKB].bitcast(BF16).rearrange(
                    "w (p s) -> p w s", p=128)
                eng.dma_start(
                    out=qT[b].rearrange("p (j s) -> p j s", j=W),
                    in_=qkv[0:Dh])
                eng.dma_start(
                    out=kT[b].rearrange("p (j s) -> p j s", j=W),
                    in_=qkv[Dh:2 * Dh])
                for sc in range(4):
                    voff = QKB + sc * 128 * Dh
                    eng.dma_start(
                        out=vS[b][:, :, 0:64].rearrange(
                            "p (w sc) d -> p sc w d", w=W)[:, sc],
                        in_=ccq_out[b][:, voff:voff + 128 * Dh].rearrange(
                            "w (p d) -> p w d", p=128))

            assemble(0)
            qkv_a2a(1)
            # FFN weights on the SWDGE (gpsimd) queues: keeps them off the
            # hardware-DMA count barriers that gate attention's first matmul
            nc.gpsimd.dma_start(out=w1_sb,
                                in_=w1_t.rearrange("(k p) n -> p k n", p=128))
            nc.gpsimd.dma_start(out=w2_sb,
                                in_=w2_t.rearrange("(m p) n -> p m n", p=128))
            nc.gpsimd.dma_start(out=b1_sb, in_=b1p.rearrange("(m p) -> p m", p=128))

            # ============ P4: causal attention (software-pipelined) ============
            if max_phase >= 4:
              with ExitStack() as ctx:
                  expp = ctx.enter_context(tc.tile_pool(name="p4e", bufs=6))
                  nrm = ctx.enter_context(tc.tile_pool(name="p4n", bufs=3))
                  scps = ctx.enter_context(tc.tile_pool(name="p4s", bufs=2, space="PSUM"))
                  atps = ctx.enter_context(tc.tile_pool(name="p4a", bufs=2, space="PSUM"))
                  # flat job list: (b, qb, pair)
                  jobs = [(b, qb, p)
                          for b in range(B) for qb in range(8)
                          for p in range(2 * (qb + 1))]
                  sc_ps = {}
                  pa_cur = {}

                  def emit_sc(job):
                      b, qb, p = job
                      q_rhs = qT[b][:, qb * 512:(qb + 1) * 512]
                      ps = scps.tile([128, 1024], F32, tag="sc", name="sc_ps_t")
                      for i in range(2):
                          kt = p * 2 + i
                          nc.tensor.matmul(ps[:, i * 512:(i + 1) * 512],
                                           kT[b][:, kt * 128:(kt + 1) * 128],
                                           q_rhs, start=True, stop=True)
                      sc_ps[job] = ps

                  def emit_pv(job):
                      b, qb, p = job
                      nkt = 4 * (qb + 1)
                      ps = sc_ps.pop(job)
                      if p == 0:
                          pa_cur[(b, qb)] = atps.tile([65, 512], F32, tag="pa",
                                                      name="pa_t")
                      pa = pa_cur[(b, qb)]
                      ex = expp.tile([128, 1024], FP8, tag="ex", name="ex_t")
                      nc.scalar.activation(out=ex, in_=ps, func=AF.Exp,
                                           scale=SCALE, bias=ln64_sb)
                      for i in range(2):
                          kt = p * 2 + i
                          jm = kt - (nkt - 4)
                          if jm >= 0:
                              nc.vector.tensor_mul(ex[:, i * 512:(i + 1) * 512],
                                                   ex[:, i * 512:(i + 1) * 512],
                                                   masks[:, jm, :])
                      nc.tensor.matmul(pa, vS[b][:, 2 * p:2 * p + 2, 0:65],
                                       ex.rearrange("c (two n) -> c two n",
                                                    two=2),
                                       start=(p == 0),
                                       stop=(p == 2 * (qb + 1) - 1),
                                       perf_mode=DR)
                      if p == 2 * (qb + 1) - 1:
                          # normalization tail for this (b, qb)
                          pa = pa_cur.pop((b, qb))
                          pa_sb = nrm.tile([65, 512], F32, tag="pasb")
                          nc.vector.tensor_copy(pa_sb, pa)
                          # reciprocal in place on partition 64, then replicate
                          # across 64 partitions on the PE (no partition-move
                          # DMA; Pool engine is busy with the next AllToAll)
                          nc.vector.reciprocal(pa_sb[64:65, :], pa_sb[64:65, :])
                          rcp_ps = p5ps.tile([Dh, 512], F32, tag="p5")
                          nc.tensor.matmul(rcp_ps, ones_row, pa_sb[64:65, :],
                                           start=True, stop=True)
                          nc.vector.tensor_mul(
                              attnT[b][:, qb * 512:(qb + 1) * 512],
                              pa_sb[0:64, :], rcp_ps)

                  # P5 for b0 token tiles overlaps b1's attention (deps via
                  # cc2_out[0], ready once the b0 AllToAll lands)
                  p5_overlap = {83: 0, 95: 1, 107: 2, 119: 3} if max_phase >= 6 \
                      else {}
                  emit_sc(jobs[0])
                  for idx, job in enumerate(jobs):
                      if idx + 1 < len(jobs):
                          emit_sc(jobs[idx + 1])
                      emit_pv(job)
                      # ship + exchange each batch as soon as it completes
                      b, qb, p = job
                      if qb == 7 and p == 2 * (qb + 1) - 1:
                          nc.sync.dma_start(
                              out=cc2_in[b].rearrange("j d i -> d j i"),
                              in_=attnT[b].rearrange("d (j i) -> d j i", j=W))
                          if not skip_cc and max_phase >= 5:
                              nc.gpsimd.collective_compute(
                                  "AllToAll", ALU.bypass,
                                  ins=[cc2_in[b].opt()], outs=[cc2_out[b].opt()],
                                  replica_groups=[list(range(W))],
                              )
                      if idx == 70:
                          assemble(1)
                      if idx in p5_overlap:
                          p5a_tile(p5_overlap[idx])
                      if idx == 125 and max_phase >= 7:
                          ln2_rsqrt(0, 4)
                      if idx in (131, 134, 137, 140) and max_phase >= 7:
                          p5b_tile(idx // 3 - 43)

        # ========== tail: P6(th0) -> P5(b1) -> P6(th1) ==========
        # P6 th0 depends only on b0's yT (done during attention), so it runs
        # on PE while the b1 AllToAll completes in the background.
        if max_phase >= 7:
          with ExitStack() as ctx:
              sb = ctx.enter_context(tc.tile_pool(name="p6", bufs=3))
              hps = ctx.enter_context(tc.tile_pool(name="p6h", bufs=2, space="PSUM"))
              o2ps = ctx.enter_context(tc.tile_pool(name="p6o", bufs=1, space="PSUM"))

              def p6_half(th):
                  po2 = [o2ps.tile([128, D], F32, tag=f"po2_{tq}", name=f"po2_{tq}")
                         for tq in range(4)]
                  for m in range(NF):
                      ph = hps.tile([128, 512], F32, tag="ph")
                      for k in range(NK):
                          nc.tensor.matmul(ph, w1_sb[:, k, m * 128:(m + 1) * 128],
                                           yT[:, k, th * 512:(th + 1) * 512],
                                           start=(k == 0), stop=(k == NK - 1))
                      hT = sb.tile([128, 512], F32R, tag="hT")
                      nc.scalar.activation(out=hT, in_=ph,
                                           func=_GELU_OVERRIDE or AF.Gelu,
                                           bias=b1_sb[:, m:m + 1])
                      for tq in range(4):
                          nc.tensor.matmul(po2[tq], hT[:, tq * 128:(tq + 1) * 128],
                                           w2_sb[:, m, :],
                                           start=(m == 0), stop=(m == NF - 1))
                  for tq in range(4):
                      t = th * 4 + tq
                      fin = sb.tile([128, D], F32, tag="fin")
                      nc.vector.tensor_add(fin, po2[tq], out1[:, t, :])
                      if has_b2:
                          nc.vector.tensor_add(fin, fin, b2_bc)
                      nc.sync.dma_start(out=out_loc[t * 128:(t + 1) * 128, :], in_=fin)

              p6_half(0)
              for t in range(4, NT):
                  p5a_tile(t)
              ln2_rsqrt(4, NT)
              for t in range(4, NT):
                  p5b_tile(t)
              p6_half(1)

        if max_phase < 7:
            with tc.tile_pool(name="dummy", bufs=1) as dp:
                dt_ = dp.tile([128, D], F32)
                nc.vector.memset(dt_, 0.0)
                for i in range(TL // 128):
                    nc.sync.dma_start(out=out_loc[i * 128:(i + 1) * 128, :], in_=dt_)
    nc.compile()
    return nc


def _prep(inputs):
    src = np.asarray(inputs["src"], np.float32)
    cos = np.asarray(inputs["rotary_cos"], np.float32).reshape(S, Dh)
    sin = np.asarray(inputs["rotary_sin"], np.float32).reshape(S, Dh)
    ipw = np.asarray(inputs["in_proj_w"], np.float32)
    ipb = np.asarray(inputs["in_proj_b"], np.float32)
    opw = np.asarray(inputs["out_proj_w"], np.float32)
    opb = np.asarray(inputs["out_proj_b"], np.float32)
    w1 = np.asarray(inputs["w1"], np.float32)
    b1 = np.asarray(inputs["b1"], np.float32)
    w2 = np.asarray(inputs["w2"], np.float32)
    b2 = np.asarray(inputs["b2"], np.float32)
    ln1_w = np.asarray(inputs["ln1_w"], np.float32)
    ln1_b = np.asarray(inputs["ln1_b"], np.float32)
    ln2_w = np.asarray(inputs["ln2_w"], np.float32)
    ln2_b = np.asarray(inputs["ln2_b"], np.float32)

    cos_full = np.tile(cos, (1, H))            # [S, D]
    sin_full = np.tile(sin, (1, H))
    d = np.arange(D)
    jj = d % Dh
    hb = d - jj
    src2 = np.where(jj < 32, hb + 2 * jj + 1, hb + 2 * (jj - 32))
    sign = np.where(jj < 32, -1.0, 1.0).astype(np.float32)
    cosw_full = ln1_w[None, :] * cos_full
    rotw_full = (sign[None, :] * ln1_w[src2][None, :]) * sin_full
    ropeb_full = (ln1_b[None, :] * cos_full
                  + (sign[None, :] * ln1_b[src2][None, :]) * sin_full)

    wq, wk, wv = ipw[0:D], ipw[D:2 * D], ipw[2 * D:3 * D]
    bq, bk, bvv = ipb[0:D], ipb[D:2 * D], ipb[2 * D:3 * D]
    # q,k packed h-major: [wq_h.T | wk_h.T] per head
    wqk_cols = []
    for h in range(H):
        wqk_cols.append(wq[h * Dh:(h + 1) * Dh].T)
        wqk_cols.append(wk[h * Dh:(h + 1) * Dh].T)
    wqk_t = np.ascontiguousarray(np.concatenate(wqk_cols, axis=1))  # [D, 1024]
    bqk_pack = np.zeros((128, H), np.float32)
    for h in range(H):
        bqk_pack[0:Dh, h] = bq[h * Dh:(h + 1) * Dh]
        bqk_pack[Dh:2 * Dh, h] = bk[h * Dh:(h + 1) * Dh]
    wv_t = np.ascontiguousarray(ln1_w[:, None] * wv.T, np.float32)  # [D, 512]
    bv_all = np.ascontiguousarray(ln1_b @ wv.T + bvv, np.float32)
    w1_t = np.ascontiguousarray(ln2_w[:, None] * w1.T, np.float32)   # [D, F]
    b1p = np.ascontiguousarray(ln2_b @ w1.T + b1, np.float32)
    wo_t = np.ascontiguousarray(opw.T)

    flags = (
        bool(np.any(ropeb_full)), bool(np.any(bq) or np.any(bk)),
        bool(np.any(bvv) or np.any(ln1_b)), bool(np.any(opb)), bool(np.any(b2)),
    )

    shared = {
        "wqk_t": wqk_t.astype(ml_dtypes.bfloat16),
        "wv_t": wv_t.astype(ml_dtypes.bfloat16),
        "bqk": bqk_pack,
        "bv": bv_all,
        "wo_t": wo_t.astype(ml_dtypes.bfloat16),
        "bo": opb,
        "w1_t": w1_t,
        "b1p": b1p,
        "w2_t": np.ascontiguousarray(w2.T),
        "b2": b2,
    }
    in_maps = []
    for c in range(W):
        m = dict(shared)
        m["src_loc"] = np.ascontiguousarray(
            src[SL * c:SL * (c + 1)].transpose(1, 0, 2).reshape(TL, D))
        m["cosw"] = np.ascontiguousarray(
            cosw_full[SL * c:SL * (c + 1)]).astype(ml_dtypes.bfloat16)
        m["rotw"] = np.ascontiguousarray(
            rotw_full[SL * c:SL * (c + 1)]).astype(ml_dtypes.bfloat16)
        if flags[0]:
            m["ropeb"] = np.ascontiguousarray(ropeb_full[SL * c:SL * (c + 1)])
        in_maps.append(m)
    return in_maps, flags


def _get_nc(flags):
    if flags not in _NC_CACHE:
        _NC_CACHE[flags] = _build_nc(flags)
    return _NC_CACHE[flags]


def kernel(**inputs):
    in_maps, flags = _prep(inputs)
    nc = _get_nc(flags)
    res = run_bass_kernel_spmd(nc, in_maps, core_ids=list(range(W)))
    out = np.empty((S, B, D), np.float32)
    for c in range(W):
        ol = res.results[c]["out_loc"].reshape(B, SL, D)
        out[SL * c:SL * (c + 1)] = ol.transpose(1, 0, 2)
    return out



# revision 22
# speedup vs baseline: 1.2038x; 1.0791x over previous
"""Trainium2 Bass kernel: transformer encoder layer (S=4096,B=2,D=512,H=8,F=2048),
causal attention + RoPE, distributed over 8 NeuronCores.

Sharding (SPMD: one program, per-core data):
  - LN1+RoPE: sequence-parallel (core c owns s in [512c, 512(c+1)), both batches)
  - QKV projections: token-parallel (each core projects q,k,v of ALL heads for
    its own tokens, emitting q/k head-major and v token-major directly)
  - AllToAll(qkv)             [384KB/rank] -> core c gets head c for all tokens
  - causal attention: head-parallel (core c owns head c, full S, both b)
  - AllToAll(attn_head^T)     [512KB/rank, per batch] -> all heads, own tokens
  - out_proj + residual + LN2 + FFN: token-parallel (core c owns its s-slice)
LayerNorm affine params are folded into downstream weights host-side.
Softmax denominators come free from a ones-column appended to V.
All weights preloaded to SBUF up front so their DMA overlaps early compute.
"""
import numpy as np
import ml_dtypes
from contextlib import ExitStack

import concourse.bass as bass
import concourse.tile as tile
from concourse import bacc, mybir
from concourse.bass_utils import run_bass_kernel_spmd
from concourse.masks import make_identity

F32 = mybir.dt.float32
F32R = mybir.dt.float32r
BF16 = mybir.dt.bfloat16
FP8 = mybir.dt.float8e4
AF = mybir.ActivationFunctionType
ALU = mybir.AluOpType
DR = mybir.MatmulPerfMode.DoubleRow
LN64 = float(np.log(64.0))  # exp bias: scales probs+denom by 64 (cancels in
                            # normalization) to keep fp8e4 values well-normal

S, B, D, H, Dh, F = 4096, 2, 512, 8, 64, 2048
W = 8                    # cores
SL = S // W              # 512 s-positions per core
TL = SL * B              # 1024 local tokens
EPS = 1e-5
SCALE = 1.0 / float(np.sqrt(Dh))  # 0.125

NT = TL // 128           # 8 local token tiles
NK = D // 128            # 4 contraction chunks over D
NF = F // 128            # 16 chunks over F
NS = S // 128            # 32 key tiles per batch
QKC = 2 * Dh * H         # 1024 packed q,k output columns (h-major)
CCW = QKC + NT * Dh      # 1536 columns per A2A block

_NC_CACHE = {}
_GELU_OVERRIDE = None  # set to AF.Identity in sim tests (CoreSim lacks Gelu)


def _layer_norm_stats(nc, pool, x_t, eps_sb):
    """Returns (rstd [128,1], negmean_rstd [128,1]) for rows of x_t."""
    stats = pool.tile([128, 6], F32, tag="st")
    nc.vector.bn_stats(out=stats, in_=x_t)
    mv = pool.tile([128, 2], F32, tag="mv")
    nc.vector.bn_aggr(out=mv, in_=stats)
    sd = pool.tile([128, 1], F32, tag="sd")
    nc.scalar.activation(out=sd, in_=mv[:, 1:2], func=AF.Sqrt, bias=eps_sb)
    rstd = pool.tile([128, 1], F32, tag="rs")
    nc.vector.reciprocal(out=rstd, in_=sd)
    nm = pool.tile([128, 1], F32, tag="nm")
    nc.vector.tensor_mul(nm, mv[:, 0:1], rstd)
    nc.vector.tensor_scalar_mul(nm, nm, -1.0)
    return rstd, nm


def _build_nc(flags, n_reps=1):
    """flags = (has_ropeb, has_bqk, has_bv, has_bo, has_b2)

    n_reps > 1 builds a timing variant with the body unrolled n_reps times
    (same I/O, idempotent) so device time can be read off the slope.
    """
    import os as _os
    has_ropeb, has_bqk, has_bv, has_bo, has_b2 = flags
    skip_cc = bool(int(_os.environ.get("K_SKIP_CC", "0")))
    max_phase = int(_os.environ.get("K_MAX_PHASE", "7"))
    nc = bacc.Bacc("TRN2", target_bir_lowering=False, debug=False, num_devices=W)

    # ---- I/O ----
    src_loc = nc.dram_tensor("src_loc", [TL, D], F32, kind="ExternalInput")
    cosw = nc.dram_tensor("cosw", [SL, D], BF16, kind="ExternalInput")
    rotw = nc.dram_tensor("rotw", [SL, D], BF16, kind="ExternalInput")
    ropeb = nc.dram_tensor("ropeb", [SL, D], F32, kind="ExternalInput") if has_ropeb else None
    wqk_t = nc.dram_tensor("wqk_t", [D, QKC], BF16, kind="ExternalInput")
    wv_t = nc.dram_tensor("wv_t", [D, D], BF16, kind="ExternalInput")
    bqk = nc.dram_tensor("bqk", [128, H], F32, kind="ExternalInput")
    bv = nc.dram_tensor("bv", [D], F32, kind="ExternalInput")
    wo_t = nc.dram_tensor("wo_t", [D, D], BF16, kind="ExternalInput")
    bo = nc.dram_tensor("bo", [D], F32, kind="ExternalInput")
    w1_t = nc.dram_tensor("w1_t", [D, F], BF16, kind="ExternalInput")
    b1p = nc.dram_tensor("b1p", [F], F32, kind="ExternalInput")
    w2_t = nc.dram_tensor("w2_t", [F, D], BF16, kind="ExternalInput")
    b2 = nc.dram_tensor("b2", [D], F32, kind="ExternalInput")
    out_loc = nc.dram_tensor("out_loc", [TL, D], F32, kind="ExternalOutput")

    with tile.TileContext(nc) as tc:
      for _rep in range(n_reps):
       with ExitStack() as top:
        dram = top.enter_context(tc.tile_pool(name="dram", bufs=1, space="DRAM"))
        consts = top.enter_context(tc.tile_pool(name="consts", bufs=1))

        # ---------- constants + all weights up front ----------
        ident = consts.tile([128, 128], F32)
        make_identity(nc, ident)
        # causal diag masks: masks[:, j, q] = 1.0 if q >= k + j*128 else 0.0
        masks_bf = consts.tile([128, 4, 512], BF16)
        for j in range(4):
            nc.gpsimd.memset(masks_bf[:, j, :], 1.0)
            nc.gpsimd.affine_select(
                out=masks_bf[:, j, :], in_=masks_bf[:, j, :],
                compare_op=ALU.is_ge, fill=0.0,
                base=-j * 128, channel_multiplier=-1, pattern=[[1, 512]],
            )
        masks = consts.tile([128, 4, 512], FP8)
        nc.vector.tensor_copy(masks, masks_bf)
        eps_sb = consts.tile([128, 1], F32)
        nc.vector.memset(eps_sb, EPS)
        ln64_sb = consts.tile([128, 1], F32)
        nc.vector.memset(ln64_sb, LN64)
        ones_col = consts.tile([65, Dh], F32)
        nc.vector.memset(ones_col, 1.0)
        ones_row = ones_col[64:65, :]   # base partition 64, matches pa_sb row
        # weight SBUF tiles declared here, DMAs emitted later (so the P1/P2'
        # token loads aren't queued behind 9MB of weight traffic)
        wqk_sb = consts.tile([128, NK, QKC], BF16)
        wv_sb = consts.tile([128, NK, D], BF16)
        wo_sb = consts.tile([128, NK, D], BF16)
        w1_sb = consts.tile([128, NK, F], BF16)
        w2_sb = consts.tile([128, NF, D], BF16)
        b1_sb = consts.tile([128, NF], F32)
        bqk_sb = consts.tile([128, H], F32)
        if has_bqk:
            nc.sync.dma_start(out=bqk_sb, in_=bqk)
        bv_bc = consts.tile([128, D], F32)
        if has_bv:
            bv_row = consts.tile([1, D], F32)
            nc.sync.dma_start(out=bv_row, in_=bv[None, :])
            nc.gpsimd.partition_broadcast(bv_bc, bv_row)
        bo_bc = consts.tile([128, D], F32)
        if has_bo:
            bo_row = consts.tile([1, D], F32)
            nc.sync.dma_start(out=bo_row, in_=bo[None, :])
            nc.gpsimd.partition_broadcast(bo_bc, bo_row)
        b2_bc = consts.tile([128, D], F32)
        if has_b2:
            b2_row = consts.tile([1, D], F32)
            nc.sync.dma_start(out=b2_row, in_=b2[None, :])
            nc.gpsimd.partition_broadcast(b2_bc, b2_row)

        # collective buffers (qkv exchange split per batch so b0's AllToAll
        # overlaps b1's LN/RoPE/projection compute). All-fp8 payload, flat per
        # dest rank: qk block [p=128][s=512], then v block [sc=4][p=128][d=64].
        # fp8 q/k feed the scores matmul directly (1 cycle/row, FWL eligible).
        QKB = 2 * Dh * SL                # 65536 qk bytes per rank
        VB = 128 * 4 * Dh                # 32768 v bytes per rank
        ccq_in = [dram.tile([W, QKB + VB], FP8, name=f"ccq_in_{b}")
                  for b in range(B)]
        ccq_out = [dram.tile([W, QKB + VB], FP8, name=f"ccq_out_{b}")
                   for b in range(B)]
        cc2_in = [dram.tile([W, Dh, SL], BF16, name=f"cc2_in_{b}") for b in range(B)]
        cc2_out = [dram.tile([W, Dh, SL], BF16, name=f"cc2_out_{b}") for b in range(B)]

        # ============ P1: LN1 + RoPE + transpose (seq-parallel) ============
        # ============ P2': QKV for own tokens, all heads ============
        with ExitStack() as ctx:
            xt_pool = ctx.enter_context(tc.tile_pool(name="xT", bufs=1))
            # D-major LN1/RoPE outputs (live through P2' only)
            xrT = xt_pool.tile([128, NK, TL], BF16)
            xnT = xt_pool.tile([128, NK, TL], BF16)
            sb = ctx.enter_context(tc.tile_pool(name="p1", bufs=3))
            small = ctx.enter_context(tc.tile_pool(name="p1s", bufs=4))
            ropet = ctx.enter_context(tc.tile_pool(name="p1r", bufs=1))
            # rope tables loaded per-sc chunk inside p1_tile (keeps the DMA
            # queue prioritized for the b0 critical path)
            cosw_all = ropet.tile([128, 4, D], BF16)
            rotw_all = ropet.tile([128, 4, D], BF16)
            trps = ctx.enter_context(tc.tile_pool(name="p1ps", bufs=2, space="PSUM"))
            qkps = ctx.enter_context(tc.tile_pool(name="p2qk", bufs=2, space="PSUM"))
            vps = ctx.enter_context(tc.tile_pool(name="p2v", bufs=2, space="PSUM"))
            stg = ctx.enter_context(tc.tile_pool(name="p2stg", bufs=3))

            def p1_tile(t):
                s_t = sb.tile([128, D], F32, tag="s")
                nc.sync.dma_start(out=s_t, in_=src_loc[t * 128:(t + 1) * 128, :])
                if t < 4:
                    nc.sync.dma_start(
                        out=cosw_all[:, t, :],
                        in_=cosw.rearrange("(s p) d -> p s d", p=128)[:, t, :])
                    nc.sync.dma_start(
                        out=rotw_all[:, t, :],
                        in_=rotw.rearrange("(s p) d -> p s d", p=128)[:, t, :])
                rstd, nm = _layer_norm_stats(nc, small, s_t, eps_sb)
                xn_t = sb.tile([128, D], F32, tag="xn")
                nc.vector.tensor_scalar(
                    out=xn_t, in0=s_t, scalar1=rstd, scalar2=nm,
                    op0=ALU.mult, op1=ALU.add,
                )
                # RoPE (ln1 affine folded into cosw/rotw/ropeb host-side)
                sc = t % (SL // 128)
                cosw_t = cosw_all[:, sc, :]
                rotw_t = rotw_all[:, sc, :]
                xr_t = sb.tile([128, D], F32, tag="xr")
                rt = sb.tile([128, D], F32, tag="rt")
                xnv = xn_t.rearrange("p (h i two) -> p h i two", h=H, two=2)
                rtv = rt.rearrange("p (h d) -> p h d", h=H)
                rwv = rotw_t.rearrange("p (h d) -> p h d", h=H)
                # rt[:, :, :32] = xn[:, :, 1::2] * rotw[:, :, :32]
                nc.vector.tensor_mul(rtv[:, :, 0:32], xnv[:, :, :, 1], rwv[:, :, 0:32])
                # rt[:, :, 32:] = xn[:, :, 0::2] * rotw[:, :, 32:]
                nc.vector.tensor_mul(rtv[:, :, 32:64], xnv[:, :, :, 0], rwv[:, :, 32:64])
                nc.vector.tensor_mul(xr_t, xn_t, cosw_t)
                nc.vector.tensor_add(xr_t, xr_t, rt)
                if has_ropeb:
                    rb_t = sb.tile([128, D], F32, tag="rb")
                    nc.sync.dma_start(out=rb_t, in_=ropeb[sc * 128:(sc + 1) * 128, :])
                    nc.vector.tensor_add(xr_t, xr_t, rb_t)
                # transpose both to D-major; xr -> DVE copy, xn -> ScalarE copy
                ps_r = trps.tile([128, 512], F32, tag="trr")
                ps_n = trps.tile([128, 512], F32, tag="trn")
                for k in range(NK):
                    nc.tensor.transpose(ps_r[:, k * 128:(k + 1) * 128],
                                        xr_t[:, k * 128:(k + 1) * 128], ident)
                    nc.tensor.transpose(ps_n[:, k * 128:(k + 1) * 128],
                                        xn_t[:, k * 128:(k + 1) * 128], ident)
                nc.vector.tensor_copy(
                    xrT[:, :, t * 128:(t + 1) * 128],
                    ps_r.rearrange("p (k i) -> p k i", k=NK))
                nc.scalar.copy(
                    out=xnT[:, :, t * 128:(t + 1) * 128],
                    in_=ps_n.rearrange("p (k i) -> p k i", k=NK))

            def p2_v(t):
                # v (all heads, token-major) for tile t -> ccq v section
                b, sc = t // 4, t % 4
                ps = vps.tile([128, D], F32, tag="v")
                for k in range(NK):
                    nc.tensor.matmul(ps, xnT[:, k, t * 128:(t + 1) * 128],
                                     wv_sb[:, k, :],
                                     start=(k == 0), stop=(k == NK - 1))
                stv = stg.tile([128, H, Dh], FP8, tag="stv")
                if has_bv:
                    nc.vector.tensor_add(
                        stv.rearrange("p h d -> p (h d)"), ps, bv_bc)
                else:
                    nc.scalar.copy(out=stv.rearrange("p h d -> p (h d)"), in_=ps)
                nc.sync.dma_start(
                    out=ccq_in[b][:, QKB:].rearrange(
                        "w (sc p d) -> sc p w d", sc=4, p=128)[sc],
                    in_=stv)

            def p2_qk(b):
                # q,k head-major for batch b -> ccq qk section (one ship DMA)
                stq = stg.tile([128, H, SL], FP8, tag="stq")
                for h in range(H):
                    ps = qkps.tile([128, SL], F32, tag="qk")
                    for k in range(NK):
                        nc.tensor.matmul(ps, wqk_sb[:, k, h * 128:(h + 1) * 128],
                                         xrT[:, k, b * SL:(b + 1) * SL],
                                         start=(k == 0), stop=(k == NK - 1))
                    if has_bqk:
                        nc.vector.tensor_scalar_add(stq[:, h, :], ps,
                                                    bqk_sb[:, h:h + 1])
                    else:
                        nc.scalar.copy(out=stq[:, h, :], in_=ps)
                nc.sync.dma_start(
                    out=ccq_in[b][:, 0:QKB].rearrange(
                        "w (p s) -> p w s", p=128),
                    in_=stq)

            def qkv_a2a(b):
                if not skip_cc and max_phase >= 2:
                    nc.gpsimd.collective_compute(
                        "AllToAll", ALU.bypass,
                        ins=[ccq_in[b].opt()], outs=[ccq_out[b].opt()],
                        replica_groups=[list(range(W))],
                    )

            p1_tile(0)
            # q/k/v weights: needed right below; emitted after tile 0's
            # src/rope DMAs so those aren't queued behind them
            nc.sync.dma_start(out=wqk_sb,
                              in_=wqk_t.rearrange("(k p) m -> p k m", p=128))
            nc.sync.dma_start(out=wv_sb,
                              in_=wv_t.rearrange("(k p) m -> p k m", p=128))
            for t in range(1, 4):
                p1_tile(t)
            for t in range(4):
                p2_v(t)
            p2_qk(0)
            qkv_a2a(0)
            for t in range(4, NT):
                p1_tile(t)
            for t in range(4, NT):
                p2_v(t)
            p2_qk(1)
            # out_proj weight: consumed by the P5a overlap during attention
            nc.sync.dma_start(out=wo_sb,
                              in_=wo_t.rearrange("(k p) n -> p k n", p=128))

        # ---- P5 resources (shared between attention overlap + tail) ----
        late = top.enter_context(tc.tile_pool(name="late", bufs=1))
        out1 = late.tile([128, NT, D], F32)   # post-attention residual stream
        yT = late.tile([128, NK, TL], BF16)    # LN2 output, D-major
        sb5 = top.enter_context(tc.tile_pool(name="p5", bufs=2))
        small5 = top.enter_context(tc.tile_pool(name="p5s", bufs=4))
        p5ps = top.enter_context(tc.tile_pool(name="p5ps", bufs=2, space="PSUM"))

        mv_all = late.tile([128, NT, 2], F32)  # LN2 mean/var per tile
        rstd_all = late.tile([128, NT], F32)
        nm_all = late.tile([128, NT], F32)
        rsq_tmp = late.tile([128, 3, NT], F32)
        rsq_i = late.tile([128, NT], mybir.dt.int32)

        def ln2_rsqrt(lo, hi):
            # Batched DVE rsqrt (quake seed + 2 Newton steps) for tiles
            # [lo, hi): keeps LN2 off ScalarE so no exp/gelu table switches.
            n = hi - lo
            xe = rsq_tmp[:, 0, lo:hi]
            xh = rsq_tmp[:, 1, lo:hi]
            nc.vector.tensor_scalar_add(xe, mv_all[:, lo:hi, 1], EPS)
            nc.vector.tensor_scalar_mul(xh, xe, -0.5)
            ib = rsq_i[:, lo:hi]
            nc.vector.tensor_scalar(out=ib, in0=xe.bitcast(mybir.dt.int32),
                                    scalar1=1, scalar2=None,
                                    op0=ALU.logical_shift_right)
            # MAGIC - (i >> 1)  ==  (~(i>>1)) + (MAGIC + 1)
            nc.vector.tensor_scalar(out=ib, in0=ib, scalar1=-1, scalar2=None,
                                    op0=ALU.bitwise_xor)
            nc.vector.tensor_scalar(out=ib, in0=ib, scalar1=0x5f3759df + 1,
                                    scalar2=None, op0=ALU.add)
            y = rstd_all[:, lo:hi]
            nc.vector.tensor_copy(y, ib.bitcast(F32))
            t2 = rsq_tmp[:, 2, lo:hi]
            for _ in range(2):
                nc.vector.tensor_mul(t2, y, y)
                nc.vector.tensor_mul(t2, t2, xh)
                nc.vector.tensor_scalar_add(t2, t2, 1.5)
                nc.vector.tensor_mul(y, y, t2)
            nc.vector.tensor_mul(nm_all[:, lo:hi], mv_all[:, lo:hi, 0], y)
            nc.vector.tensor_scalar_mul(nm_all[:, lo:hi], nm_all[:, lo:hi], -1.0)

        def p5a_tile(t):
            # out_proj + residual + LN2 stats (PE/DVE only — safe to overlap
            # attention without touching ScalarE's loaded exp table set)
            b, sc = t // (NT // B), t % (NT // B)
            po = p5ps.tile([128, D], F32, tag="p5")
            a_sb = sb5.tile([128, NK, 128], BF16, tag="a")
            nc.gpsimd.dma_start(
                out=a_sb,
                in_=cc2_out[b][:, :, sc * 128:(sc + 1) * 128].rearrange(
                    "(k e) d i -> (e d) k i", k=NK))
            for k in range(NK):
                nc.tensor.matmul(po, a_sb[:, k, :], wo_sb[:, k, :],
                                 start=(k == 0), stop=(k == NK - 1))
            s_t = sb5.tile([128, D], F32, tag="s")
            nc.sync.dma_start(out=s_t, in_=src_loc[t * 128:(t + 1) * 128, :])
            o1 = out1[:, t, :]
            nc.vector.tensor_add(o1, po, s_t)
            if has_bo:
                nc.vector.tensor_add(o1, o1, bo_bc)
            stats = small5.tile([128, 6], F32, tag="st")
            nc.vector.bn_stats(out=stats, in_=o1)
            nc.vector.bn_aggr(out=mv_all[:, t, :], in_=stats)

        def p5b_tile(t):
            # LN2 normalize + yT transpose (rstd/nm precomputed on DVE)
            y_t = sb5.tile([128, D], F32, tag="y")
            nc.vector.tensor_scalar(out=y_t, in0=out1[:, t, :],
                                    scalar1=rstd_all[:, t:t + 1],
                                    scalar2=nm_all[:, t:t + 1],
                                    op0=ALU.mult, op1=ALU.add)
            ps = p5ps.tile([128, 512], F32, tag="p5")
            for k in range(NK):
                nc.tensor.transpose(ps[:, k * 128:(k + 1) * 128],
                                    y_t[:, k * 128:(k + 1) * 128], ident)
            # DVE, not ScalarE: the overlapped p5b tiles run while attention
            # saturates ScalarE with exp
            nc.vector.tensor_copy(
                yT[:, :, t * 128:(t + 1) * 128],
                ps.rearrange("p (k i) -> p k i", k=NK))

        with ExitStack() as actx:
          if max_phase >= 3:
            act = actx.enter_context(tc.tile_pool(name="act", bufs=1))
            # per-batch tiles: batch b's attention must not depend on the
            # other batch's AllToAll (dep tracking is tile-granular).
            qT = [act.tile([Dh, S], FP8, name=f"qT{b}") for b in range(B)]
            kT = [act.tile([Dh, S], FP8, name=f"kT{b}") for b in range(B)]
            # vS row stride 80 (not 65): DoubleRow needs the k-tile step to be
            # a multiple of 16 bytes. col 64 = ones (softmax denominator).
            vS = [act.tile([128, NS, 80], FP8, name=f"vS{b}") for b in range(B)]
            attnT = [act.tile([Dh, S], BF16, name=f"attnT{b}") for b in range(B)]
            for b in range(B):
                nc.vector.memset(vS[b][:, :, 64:65], 1.0)

            # ---- assembly: head c = my rank's block. b0's assembly is
            # emitted BEFORE the b1 AllToAll: deps collapse to per-engine
            # counters, so anything emitted after the second collective
            # waits for both. ----
            def assemble(b):
                eng = nc.sync if b == 0 else nc.gpsimd
                qkv = ccq_out[b][:, 0:QKB].rearrange(
                    "w (p s) -> p w s", p=128)
                eng.dma_start(
                    out=qT[b].rearrange("p (j s) -> p j s", j=W),
                    in_=qkv[0:Dh])
                eng.dma_start(
                    out=kT[b].rearrange("p (j s) -> p j s", j=W),
                    in_=qkv[Dh:2 * Dh])
                for sc in range(4):
                    voff = QKB + sc * 128 * Dh
                    eng.dma_start(
                        out=vS[b][:, :, 0:64].rearrange(
                            "p (w sc) d -> p sc w d", w=W)[:, sc],
                        in_=ccq_out[b][:, voff:voff + 128 * Dh].rearrange(
                            "w (p d) -> p w d", p=128))

            assemble(0)
            qkv_a2a(1)
            # FFN weights on the SWDGE (gpsimd) queues: keeps them off the
            # hardware-DMA count barriers that gate attention's first matmul
            nc.gpsimd.dma_start(out=w1_sb,
                                in_=w1_t.rearrange("(k p) n -> p k n", p=128))
            nc.gpsimd.dma_start(out=w2_sb,
                                in_=w2_t.rearrange("(m p) n -> p m n", p=128))
            nc.gpsimd.dma_start(out=b1_sb, in_=b1p.rearrange("(m p) -> p m", p=128))

            # ============ P4: causal attention (software-pipelined) ============
            if max_phase >= 4:
              with ExitStack() as ctx:
                  expp = ctx.enter_context(tc.tile_pool(name="p4e", bufs=6))
                  nrm = ctx.enter_context(tc.tile_pool(name="p4n", bufs=3))
                  scps = ctx.enter_context(tc.tile_pool(name="p4s", bufs=2, space="PSUM"))
                  atps = ctx.enter_context(tc.tile_pool(name="p4a", bufs=2, space="PSUM"))
                  # flat job list: (b, qb, pair)
                  jobs = [(b, qb, p)
                          for b in range(B) for qb in range(8)
                          for p in range(2 * (qb + 1))]
                  sc_ps = {}
                  pa_cur = {}

                  def emit_sc(job):
                      b, qb, p = job
                      q_rhs = qT[b][:, qb * 512:(qb + 1) * 512]
                      ps = scps.tile([128, 1024], F32, tag="sc", name="sc_ps_t")
                      for i in range(2):
                          kt = p * 2 + i
                          nc.tensor.matmul(ps[:, i * 512:(i + 1) * 512],
                                           kT[b][:, kt * 128:(kt + 1) * 128],
                                           q_rhs, start=True, stop=True)
                      sc_ps[job] = ps

                  def emit_pv(job):
                      b, qb, p = job
                      nkt = 4 * (qb + 1)
                      ps = sc_ps.pop(job)
                      if p == 0:
                          pa_cur[(b, qb)] = atps.tile([65, 512], F32, tag="pa",
                                                      name="pa_t")
                      pa = pa_cur[(b, qb)]
                      ex = expp.tile([128, 1024], FP8, tag="ex", name="ex_t")
                      nc.scalar.activation(out=ex, in_=ps, func=AF.Exp,
                                           scale=SCALE, bias=ln64_sb)
                      for i in range(2):
                          kt = p * 2 + i
                          jm = kt - (nkt - 4)
                          if jm >= 0:
                              nc.vector.tensor_mul(ex[:, i * 512:(i + 1) * 512],
                                                   ex[:, i * 512:(i + 1) * 512],
                                                   masks[:, jm, :])
                      nc.tensor.matmul(pa, vS[b][:, 2 * p:2 * p + 2, 0:65],
                                       ex.rearrange("c (two n) -> c two n",
                                                    two=2),
                                       start=(p == 0),
                                       stop=(p == 2 * (qb + 1) - 1),
                                       perf_mode=DR)
                      if p == 2 * (qb + 1) - 1:
                          # normalization tail for this (b, qb)
                          pa = pa_cur.pop((b, qb))
                          pa_sb = nrm.tile([65, 512], F32, tag="pasb")
                          nc.vector.tensor_copy(pa_sb, pa)
                          # reciprocal in place on partition 64, then replicate
                          # across 64 partitions on the PE (no partition-move
                          # DMA; Pool engine is busy with the next AllToAll)
                          nc.vector.reciprocal(pa_sb[64:65, :], pa_sb[64:65, :])
                          rcp_ps = p5ps.tile([Dh, 512], F32, tag="p5")
                          nc.tensor.matmul(rcp_ps, ones_row, pa_sb[64:65, :],
                                           start=True, stop=True)
                          nc.vector.tensor_mul(
                              attnT[b][:, qb * 512:(qb + 1) * 512],
                              pa_sb[0:64, :], rcp_ps)

                  # P5 for b0 token tiles overlaps b1's attention (deps via
                  # cc2_out[0], ready once the b0 AllToAll lands)
                  p5_overlap = {108: 0, 116: 1, 124: 2, 130: 3} \
                      if max_phase >= 6 else {}
                  emit_sc(jobs[0])
                  for idx, job in enumerate(jobs):
                      if idx + 1 < len(jobs):
                          emit_sc(jobs[idx + 1])
                      emit_pv(job)
                      # ship + exchange each batch as soon as it completes
                      b, qb, p = job
                      if qb == 7 and p == 2 * (qb + 1) - 1:
                          nc.sync.dma_start(
                              out=cc2_in[b].rearrange("j d i -> d j i"),
                              in_=attnT[b].rearrange("d (j i) -> d j i", j=W))
                          if not skip_cc and max_phase >= 5:
                              nc.gpsimd.collective_compute(
                                  "AllToAll", ALU.bypass,
                                  ins=[cc2_in[b].opt()], outs=[cc2_out[b].opt()],
                                  replica_groups=[list(range(W))],
                              )
                      if idx == 58:
                          assemble(1)
                      if idx in p5_overlap:
                          p5a_tile(p5_overlap[idx])
                      if idx == 134 and max_phase >= 7:
                          ln2_rsqrt(0, 4)
                      if idx in (137, 139, 141, 143) and max_phase >= 7:
                          p5b_tile((idx - 137) // 2)

        # ========== tail: P6(th0) -> P5(b1) -> P6(th1) ==========
        # P6 th0 depends only on b0's yT (done during attention), so it runs
        # on PE while the b1 AllToAll completes in the background.
        if max_phase >= 7:
          with ExitStack() as ctx:
              sb = ctx.enter_context(tc.tile_pool(name="p6", bufs=3))
              hps = ctx.enter_context(tc.tile_pool(name="p6h", bufs=2, space="PSUM"))
              o2ps = ctx.enter_context(tc.tile_pool(name="p6o", bufs=1, space="PSUM"))

              def p6_half(th):
                  po2 = [o2ps.tile([128, D], F32, tag=f"po2_{tq}", name=f"po2_{tq}")
                         for tq in range(4)]
                  for m in range(NF):
                      ph = hps.tile([128, 512], F32, tag="ph")
                      for k in range(NK):
                          nc.tensor.matmul(ph, w1_sb[:, k, m * 128:(m + 1) * 128],
                                           yT[:, k, th * 512:(th + 1) * 512],
                                           start=(k == 0), stop=(k == NK - 1))
                      hT = sb.tile([128, 512], BF16, tag="hT")
                      nc.scalar.activation(out=hT, in_=ph,
                                           func=_GELU_OVERRIDE or AF.Gelu,
                                           bias=b1_sb[:, m:m + 1])
                      for tq in range(4):
                          nc.tensor.matmul(po2[tq], hT[:, tq * 128:(tq + 1) * 128],
                                           w2_sb[:, m, :],
                                           start=(m == 0), stop=(m == NF - 1))
                  for tq in range(4):
                      t = th * 4 + tq
                      fin = sb.tile([128, D], F32, tag="fin")
                      nc.vector.tensor_add(fin, po2[tq], out1[:, t, :])
                      if has_b2:
                          nc.vector.tensor_add(fin, fin, b2_bc)
                      nc.sync.dma_start(out=out_loc[t * 128:(t + 1) * 128, :], in_=fin)

              p6_half(0)
              for t in range(4, NT):
                  p5a_tile(t)
              ln2_rsqrt(4, NT)
              for t in range(4, NT):
                  p5b_tile(t)
              p6_half(1)

        if max_phase < 7:
            with tc.tile_pool(name="dummy", bufs=1) as dp:
                dt_ = dp.tile([128, D], F32)
                nc.vector.memset(dt_, 0.0)
                for i in range(TL // 128):
                    nc.sync.dma_start(out=out_loc[i * 128:(i + 1) * 128, :], in_=dt_)
    nc.compile()
    return nc


def _prep(inputs):
    src = np.asarray(inputs["src"], np.float32)
    cos = np.asarray(inputs["rotary_cos"], np.float32).reshape(S, Dh)
    sin = np.asarray(inputs["rotary_sin"], np.float32).reshape(S, Dh)
    ipw = np.asarray(inputs["in_proj_w"], np.float32)
    ipb = np.asarray(inputs["in_proj_b"], np.float32)
    opw = np.asarray(inputs["out_proj_w"], np.float32)
    opb = np.asarray(inputs["out_proj_b"], np.float32)
    w1 = np.asarray(inputs["w1"], np.float32)
    b1 = np.asarray(inputs["b1"], np.float32)
    w2 = np.asarray(inputs["w2"], np.float32)
    b2 = np.asarray(inputs["b2"], np.float32)
    ln1_w = np.asarray(inputs["ln1_w"], np.float32)
    ln1_b = np.asarray(inputs["ln1_b"], np.float32)
    ln2_w = np.asarray(inputs["ln2_w"], np.float32)
    ln2_b = np.asarray(inputs["ln2_b"], np.float32)

    cos_full = np.tile(cos, (1, H))            # [S, D]
    sin_full = np.tile(sin, (1, H))
    d = np.arange(D)
    jj = d % Dh
    hb = d - jj
    src2 = np.where(jj < 32, hb + 2 * jj + 1, hb + 2 * (jj - 32))
    sign = np.where(jj < 32, -1.0, 1.0).astype(np.float32)
    cosw_full = ln1_w[None, :] * cos_full
    rotw_full = (sign[None, :] * ln1_w[src2][None, :]) * sin_full
    ropeb_full = (ln1_b[None, :] * cos_full
                  + (sign[None, :] * ln1_b[src2][None, :]) * sin_full)

    wq, wk, wv = ipw[0:D], ipw[D:2 * D], ipw[2 * D:3 * D]
    bq, bk, bvv = ipb[0:D], ipb[D:2 * D], ipb[2 * D:3 * D]
    # q,k packed h-major: [wq_h.T | wk_h.T] per head
    wqk_cols = []
    for h in range(H):
        wqk_cols.append(wq[h * Dh:(h + 1) * Dh].T)
        wqk_cols.append(wk[h * Dh:(h + 1) * Dh].T)
    wqk_t = np.ascontiguousarray(np.concatenate(wqk_cols, axis=1))  # [D, 1024]
    bqk_pack = np.zeros((128, H), np.float32)
    for h in range(H):
        bqk_pack[0:Dh, h] = bq[h * Dh:(h + 1) * Dh]
        bqk_pack[Dh:2 * Dh, h] = bk[h * Dh:(h + 1) * Dh]
    wv_t = np.ascontiguousarray(ln1_w[:, None] * wv.T, np.float32)  # [D, 512]
    bv_all = np.ascontiguousarray(ln1_b @ wv.T + bvv, np.float32)
    w1_t = np.ascontiguousarray(ln2_w[:, None] * w1.T, np.float32)   # [D, F]
    b1p = np.ascontiguousarray(ln2_b @ w1.T + b1, np.float32)
    wo_t = np.ascontiguousarray(opw.T)

    flags = (
        bool(np.any(ropeb_full)), bool(np.any(bq) or np.any(bk)),
        bool(np.any(bvv) or np.any(ln1_b)), bool(np.any(opb)), bool(np.any(b2)),
    )

    shared = {
        "wqk_t": wqk_t.astype(ml_dtypes.bfloat16),
        "wv_t": wv_t.astype(ml_dtypes.bfloat16),
        "bqk": bqk_pack,
        "bv": bv_all,
        "wo_t": wo_t.astype(ml_dtypes.bfloat16),
        "bo": opb,
        "w1_t": w1_t.astype(ml_dtypes.bfloat16),
        "b1p": b1p,
        "w2_t": np.ascontiguousarray(w2.T).astype(ml_dtypes.bfloat16),
        "b2": b2,
    }
    in_maps = []
    for c in range(W):
        m = dict(shared)
        m["src_loc"] = np.ascontiguousarray(
            src[SL * c:SL * (c + 1)].transpose(1, 0, 2).reshape(TL, D))
        m["cosw"] = np.ascontiguousarray(
            cosw_full[SL * c:SL * (c + 1)]).astype(ml_dtypes.bfloat16)
        m["rotw"] = np.ascontiguousarray(
            rotw_full[SL * c:SL * (c + 1)]).astype(ml_dtypes.bfloat16)
        if flags[0]:
            m["ropeb"] = np.ascontiguousarray(ropeb_full[SL * c:SL * (c + 1)])
        in_maps.append(m)
    return in_maps, flags


def _get_nc(flags):
    if flags not in _NC_CACHE:
        _NC_CACHE[flags] = _build_nc(flags)
    return _NC_CACHE[flags]


def kernel(**inputs):
    in_maps, flags = _prep(inputs)
    nc = _get_nc(flags)
    res = run_bass_kernel_spmd(nc, in_maps, core_ids=list(range(W)))
    out = np.empty((S, B, D), np.float32)
    for c in range(W):
        ol = res.results[c]["out_loc"].reshape(B, SL, D)
        out[SL * c:SL * (c + 1)] = ol.transpose(1, 0, 2)
    return out

